# revision 8
# baseline (speedup 1.0000x reference)
"""Trainium2 Bass kernel for a codec-transformer block (sliding-window GQA + SwiGLU).

Sharding: data-parallel over 8 token chunks (2 batches x 4 chunks of 512
tokens). The 512-token sliding window makes attention local: each core
receives its 512 "own" tokens plus the preceding 512 tokens as a KV halo,
so no collectives are needed.

Host-side prep (layout only, no model FLOPs):
  - attn_norm_w folded into wq/wk/wv columns, ffn_norm_w into w1/w3 columns
  - attn_scale/ffn_scale: their scalar parts are applied on-chip (c_wo, c_y)
    so the fp8 weights keep a healthy range; only the shape (ratio to the
    scalar) is folded into wo/w2 rows
  - wq/wkv/wo/w1/w3/w2 are cast to fp8e4 with power-of-two range scales;
    every scale is absorbed into an op the kernel already needs:
      * wq,wk x32: cancels in qk-rmsnorm (scale-invariant)
      * wv x32: undone by the per-token rstd fold at the V eviction
      * w1,w3 x8: undone by scaling hn by 1/8 (folded into the hn-rmsnorm
        sqrt scale), making psg/psu exact so fT = silu(psg)*psu is exact
      * wo x16 / w2 x16: undone by the c_wo/c_y constants on the h/y paths
  - x is sent twice: bf16 [CTX,D] for the transpose/matmul path and f32
    [OWN,D] for the residual; the x-rmsnorm cancels in qk-norm for Q/K and
    is applied to V via rstd at its eviction, so raw x feeds the PE
    transposes directly (no norm on the critical path).

Attention stage: head pairs (sharing a kv pair-transposed kT2 tile) run
concurrently on PE row groups 0-63/64-127. Scores for one head live in two
[P,1280] PSUM tiles with a ragged ki-permuted layout chosen so every
matmul output stays inside a 2KB bank; exp then runs as ONE activation per
half-head. The sliding-window mask reduces to two constant 128x128
triangles applied only to the two diagonal blocks per query tile
(copy_predicated with zeros); halo-padding tokens are excluded via a
0/1 validity column in V's appended ones-column, which zeroes both their
numerator and softmax-denominator contributions.
"""

import os
import sys

sys.path.insert(0, "/opt/trn_rl_repo")
os.environ.setdefault("MYCRO_LOCAL_CACHE", "1")

from contextlib import ExitStack

import numpy as np
import ml_dtypes

import concourse.bass as bass
import concourse.bacc as bacc
import concourse.tile as tile
from concourse import mybir
from concourse.masks import make_identity
from concourse.bass_utils import run_bass_kernel_spmd

BF16 = mybir.dt.bfloat16
F32 = mybir.dt.float32
FP8 = mybir.dt.float8e4
AF = mybir.ActivationFunctionType
DR = mybir.MatmulPerfMode.DoubleRow
NPBF16 = ml_dtypes.bfloat16
NPFP8 = ml_dtypes.float8_e4m3

P = 128
B, T, D = 2, 2048, 1024
HID = 4096
H, KVH, HD = 16, 4, 64
KD = D // P            # 8 contraction tiles over model dim
KH = HID // P          # 32 contraction tiles over hidden dim
OWN = 512              # tokens owned per core
CTX = 1024             # own + 512-token halo
NQT = OWN // P         # 4
NKT = CTX // P         # 8
NCORES = 8
KC = KVH * HD          # 256
EPS = 1e-5
QKEPS = 1e-6
SM_SCALE = 1.0 / 8.0   # 1/sqrt(HD)

S_WQKV = 32.0          # fp8 range scale on wq/wk/wv
S_W13 = 8.0            # fp8 range scale on w1/w3 (alpha = 1/8 on hn)
S_W2 = 16.0            # fp8 range scale on w2
S_WO = 16.0            # fp8 range scale on wo

# Ragged in-bank PSUM layout for one half-head of scores ([P,1280] f32).
# Widths per ki: 128,256,384,512,512,384,256,128; this permutation keeps
# every matmul output inside a 2KB (512-f32) PSUM bank.
OFF = {0: 896, 1: 1024, 2: 512, 3: 0,
       4: 1280 + 0, 5: 1280 + 512, 6: 1280 + 1024, 7: 1280 + 896}


def _qclip(ki):
    """Valid own-query range for ctx key tile ki under the sliding window."""
    return max(0, P * (ki - 4)), min(OWN, P * (ki + 1))


def _es_col(qt, ki):
    """eS/psum column of query-tile block (qt, ki) in the ragged layout."""
    return OFF[ki] + qt * P - _qclip(ki)[0]


def _build_tile_kernel(ctx: ExitStack, tc: tile.TileContext, io: dict):
    nc = tc.nc
    y = io["y"]

    const = ctx.enter_context(tc.tile_pool(name="const", bufs=1))
    identity = const.tile([P, P], BF16)
    make_identity(nc, identity)
    qw2_sb = const.tile([P, 1], F32)    # q_norm_w tiled over both 64-rows
    nc.sync.dma_start(qw2_sb, io["qw2"])
    kw2_sb = const.tile([P, 1], F32)
    nc.sync.dma_start(kw2_sb, io["kw2"])
    tri_g = const.tile([P, P], BF16)        # d0 VALID mask (k > qq), gpsimd
    nc.sync.dma_start(tri_g, io["tri_g"])
    tri_p = const.tile([P, P], mybir.dt.uint8)  # d4 INVALID mask (k > qq), DVE
    nc.sync.dma_start(tri_p, io["tri_p"])
    zeros_sb = const.tile([P, P], BF16)
    nc.vector.memset(zeros_sb, 0.0)
    vm_sb = const.tile([P, NKT], BF16)      # per-token validity (halo pad=0)
    nc.sync.dma_start(vm_sb, io["vones"])
    epsv_sb = const.tile([P, 1], F32)
    nc.vector.memset(epsv_sb, EPS * S_WQKV * S_WQKV)
    epsh_sb = const.tile([P, 1], F32)
    nc.vector.memset(epsh_sb, EPS * S_W13 * S_W13)
    qkeps_sb = const.tile([P, 1], F32)
    nc.vector.memset(qkeps_sb, QKEPS)

    sstat = ctx.enter_context(tc.tile_pool(name="sstat", bufs=8))

    pers = ctx.enter_context(tc.tile_pool(name="pers", bufs=1))
    h_sb = pers.tile([P, NQT, D], F32)       # residual h = x + r, fp32
    hnT_pool = ctx.enter_context(tc.tile_pool(name="hnT_pool", bufs=1))
    hnT = hnT_pool.tile([P, KD, OWN], FP8)
    ap_stack = ExitStack()
    attn_pers = ap_stack.enter_context(tc.tile_pool(name="attn_pers", bufs=1))
    # qhat^T: q heads are laid out (via the host-side wq column permutation)
    # so head h lives in feature tile tau=(h%4)+4*(h//8) at partition base
    # pi=((h//4)%2)*64 -- exactly where its kv head lands in kT2's natural
    # pair-transpose layout, so scores operands always share a base partition.
    qkT = attn_pers.tile([P, KD, OWN], BF16)
    kT2 = attn_pers.tile([P, 2, CTX], BF16)
    v65 = attn_pers.tile([P, NKT, KVH, HD + 1], BF16)  # v tokens + valid col
    attn_sb = attn_pers.tile([P, NQT, H * HD], BF16)  # attn out, token-major
    for kvh in range(KVH):
        nc.vector.tensor_copy(v65[:, :, kvh, HD:HD + 1], vm_sb[:, :, None])

    xall_stack = ExitStack()
    xall_pool = xall_stack.enter_context(tc.tile_pool(name="xall", bufs=1))
    xbf = xall_pool.tile([P, NKT, D], BF16)
    xown = xall_pool.tile([P, NQT, D], F32)
    xT = xall_pool.tile([P, KD, CTX], FP8)
    for kd in range(KD):
        nc.sync.dma_start(xT[:, kd, :],
                          io["xTf8"][kd * P:(kd + 1) * P, :])
    for i in range(NKT):
        nc.sync.dma_start(xbf[:, i, :], io["xbf"][i * P:(i + 1) * P, :])
    for i in range(NQT):
        nc.sync.dma_start(xown[:, i, :], io["xown"][i * P:(i + 1) * P, :])

    # ---- Stages A+B: x transpose + QKV (fp8 DoubleRow), per ctx tile ----
    stage_a = ExitStack()
    with stage_a:
        wqkv_pool = stage_a.enter_context(tc.tile_pool(name="wqkv", bufs=1))
        wkv_sb = wqkv_pool.tile([P, KD, 2 * KC], FP8)
        nc.sync.dma_start(wkv_sb, io["wkvT"].rearrange("(kd p) n -> p kd n", p=P))
        wq_sb = wqkv_pool.tile([P, KD, D], FP8)
        nc.sync.dma_start(wq_sb, io["wqT"].rearrange("(kd p) n -> p kd n", p=P))

        tp_ps = stage_a.enter_context(
            tc.tile_pool(name="tp_ps", bufs=3, space="PSUM"))
        pb_ps = stage_a.enter_context(
            tc.tile_pool(name="pb_ps", bufs=5, space="PSUM"))
        pb = stage_a.enter_context(tc.tile_pool(name="pb", bufs=3))

        def stats_tile(i):
            """sum(x^2) -> rstd_v = 1/(32*sqrt(mean+eps)) (ACT/DVE only)."""
            sq = pb.tile([P, D], F32, tag="sq")
            ssq = sstat.tile([P, 1], F32, tag="ssq")
            nc.scalar.activation(sq, xbf[:, i, :], AF.Square, accum_out=ssq)
            stdv = sstat.tile([P, 1], F32, tag="std")
            nc.scalar.activation(stdv, ssq, AF.Sqrt, bias=epsv_sb,
                                 scale=S_WQKV * S_WQKV / D)
            rstd_v = sstat.tile([P, 1], F32, tag="rstdv")
            nc.vector.reciprocal(rstd_v, stdv)
            return rstd_v

        def emit_k_tp(kt, khat):
            # eviction applies k_norm_w (per feature = per partition here)
            pt = tp_ps.tile([P, 2, P], BF16, tag="tp")
            for kf in range(2):
                nc.tensor.transpose(pt[:, kf, :],
                                    khat[:, kf * P:(kf + 1) * P], identity)
            nc.vector.tensor_scalar_mul(
                kT2[:, :, kt * P:(kt + 1) * P], pt, kw2_sb)

        def emit_q_tp(qt, qhats):
            for half in range(2):
                for j in range(0, 4, 2):
                    pt = tp_ps.tile([P, 2, P], BF16, tag="tp")
                    nc.tensor.transpose(
                        pt[:, 0, :], qhats[half][:, j * P:(j + 1) * P],
                        identity)
                    nc.tensor.transpose(
                        pt[:, 1, :], qhats[half][:, (j + 1) * P:(j + 2) * P],
                        identity)
                    nc.vector.tensor_scalar_mul(
                        qkT[:, half * 4 + j:half * 4 + j + 2,
                            qt * P:(qt + 1) * P], pt, qw2_sb)

        # kq-hat transposes run TWO tiles behind their matmuls so the
        # qk-norm ACT/DVE chains never stall the PE stream.
        rstds = {0: stats_tile(0), 1: stats_tile(1)}
        pend_k = {}
        pend_q = {}
        for i in range(NKT):
            rstd_v = rstds.pop(i)
            # K / V projection for ctx tile i (fp8 DoubleRow over kd pairs)
            ps = pb_ps.tile([P, 512], F32, tag="ps")
            for j in range(KD // 2):
                nc.tensor.matmul(
                    ps, lhsT=xT[:, 2 * j:2 * j + 2, i * P:(i + 1) * P],
                    rhs=wkv_sb[:, 2 * j:2 * j + 2, :],
                    start=(j == 0), stop=(j == KD // 2 - 1), perf_mode=DR)
            kv_ps = ps

            # Q projection for own tile qt = i - 4
            q_pss = None
            if i >= NQT:
                qt = i - NQT
                col = OWN + qt * P
                q_pss = []
                for half in range(2):
                    ps = pb_ps.tile([P, 512], F32, tag="ps")
                    q_pss.append(ps)
                    for j in range(KD // 2):
                        nc.tensor.matmul(
                            ps, lhsT=xT[:, 2 * j:2 * j + 2, col:col + P],
                            rhs=wq_sb[:, 2 * j:2 * j + 2,
                                      half * 512:(half + 1) * 512],
                            start=(j == 0), stop=(j == KD // 2 - 1),
                            perf_mode=DR)

            # two-behind transposes keep the PE stream dense
            if i - 2 in pend_k:
                emit_k_tp(i - 2, pend_k.pop(i - 2))
            if i - 2 - NQT in pend_q:
                emit_q_tp(i - 2 - NQT, pend_q.pop(i - 2 - NQT))

            # k-chain + v eviction (ACT/DVE)
            ps = kv_ps
            sqk = pb.tile([P, KC], F32, tag="sqk")
            nc.scalar.activation(sqk, ps[:, 0:KC], AF.Square)
            msk = pb.tile([P, KVH], F32, tag="msk")
            nc.vector.reduce_sum(
                msk, sqk.rearrange("p (h e) -> p h e", e=HD),
                axis=mybir.AxisListType.X)
            sck = sstat.tile([P, KVH], F32, tag="sck")
            nc.scalar.activation(sck, msk, AF.Sqrt, bias=qkeps_sb, scale=1.0 / HD)
            rck = sstat.tile([P, KVH], F32, tag="rck")
            nc.vector.reciprocal(rck, sck)
            khat = pb.tile([P, KC], BF16, tag="khat")
            nc.vector.tensor_mul(
                khat.rearrange("p (h e) -> p h e", e=HD),
                ps[:, 0:KC].rearrange("p (h e) -> p h e", e=HD),
                rck[:, :, None].broadcast_to([P, KVH, HD]))
            pend_k[i] = khat
            nc.vector.tensor_scalar_mul(
                v65[:, i, :, 0:HD],
                ps[:, KC:2 * KC].rearrange("p (h e) -> p h e", e=HD),
                rstd_v)

            # q-chain
            if q_pss is not None:
                qt = i - NQT
                msq = pb.tile([P, H], F32, tag="msq")
                for half in range(2):
                    sqq = pb.tile([P, 512], F32, tag="sqq")
                    nc.scalar.activation(sqq, q_pss[half], AF.Square)
                    nc.vector.reduce_sum(
                        msq[:, half * 8:(half + 1) * 8],
                        sqq.rearrange("p (h e) -> p h e", e=HD),
                        axis=mybir.AxisListType.X)
                sc = sstat.tile([P, H], F32, tag="sc")
                nc.scalar.activation(sc, msq, AF.Sqrt, bias=qkeps_sb,
                                     scale=1.0 / HD)
                rc = sstat.tile([P, H], F32, tag="rc")
                nc.vector.reciprocal(rc, sc)
                qhats = []
                for half in range(2):
                    ps = q_pss[half]
                    qhat = pb.tile([P, 512], BF16, tag="qhat")
                    nc.vector.tensor_mul(
                        qhat.rearrange("p (h e) -> p h e", e=HD),
                        ps.rearrange("p (h e) -> p h e", e=HD),
                        rc[:, half * 8:(half + 1) * 8, None]
                        .broadcast_to([P, 8, HD]))
                    qhats.append(qhat)
                pend_q[qt] = qhats

            if i + 2 < NKT:
                rstds[i + 2] = stats_tile(i + 2)

        for i in (NKT - 2, NKT - 1):
            emit_k_tp(i, pend_k.pop(i))
        for qt in (NQT - 2, NQT - 1):
            emit_q_tp(qt, pend_q.pop(qt))

    # ---- Stage C: attention. Head pairs run on PE row groups 0/64. ----
    stage_c = ExitStack()
    with stage_c:
        es_pool = stage_c.enter_context(tc.tile_pool(name="es_pool", bufs=2))
        psc = stage_c.enter_context(
            tc.tile_pool(name="psc", bufs=1, space="PSUM"))
        ps_o = stage_c.enter_context(
            tc.tile_pool(name="ps_o", bufs=2, space="PSUM"))

        def emit_pv(h, eS, eSd):
            kvh = h // 4
            tau = (h % 4) + 4 * (h // 8)
            pi = ((h // 4) % 2)
            slot = 2 * tau + pi
            for qt in range(NQT):
                po = ps_o.tile([P, HD + 1], F32, tag="po")
                for j in range(5):
                    if j == 0:
                        lhs = eSd[:, qt, :]
                    else:
                        c = _es_col(qt, qt + j)
                        lhs = eS[:, c:c + P]
                    nc.tensor.matmul(
                        po, lhsT=lhs,
                        rhs=v65[:, qt + j, kvh, :],
                        start=(j == 0), stop=(j == 4))
                rec = sstat.tile([P, 1], F32, tag="rec")
                nc.vector.reciprocal(rec, po[:, HD:HD + 1])
                nc.vector.tensor_scalar_mul(
                    attn_sb[:, qt, slot * HD:(slot + 1) * HD], po[:, 0:HD],
                    rec)

        PAIRS = [(0, 4), (1, 5), (2, 6), (3, 7),
                 (8, 12), (9, 13), (10, 14), (11, 15)]
        pending = []
        for hA, hB in PAIRS:
            g = hA // 8
            tau = (hA % 4) + 4 * (hA // 8)
            eS_A = es_pool.tile([P, 2 * 1280], BF16, tag="esA")
            eS_B = es_pool.tile([P, 2 * 1280], BF16, tag="esB")
            eSd_A = es_pool.tile([P, NQT, P], BF16, tag="esdA")
            eSd_B = es_pool.tile([P, NQT, P], BF16, tag="esdB")
            for half in range(2):
                psA = psc.tile([P, 1280], F32, tag="psA")
                psB = psc.tile([P, 1280], F32, tag="psB")
                for ki in range(half * 4, half * 4 + 4):
                    qlo, qhi = _qclip(ki)
                    w = qhi - qlo
                    o = OFF[ki] - half * 1280
                    nc.tensor.matmul(
                        psA[:, o:o + w],
                        lhsT=kT2[0:HD, g, ki * P:(ki + 1) * P],
                        rhs=qkT[0:HD, tau, qlo:qhi],
                        start=True, stop=True)
                    nc.tensor.matmul(
                        psB[:, o:o + w],
                        lhsT=kT2[HD:P, g, ki * P:(ki + 1) * P],
                        rhs=qkT[HD:P, tau, qlo:qhi],
                        start=True, stop=True)
                for eS, psX in ((eS_A, psA), (eS_B, psB)):
                    nc.scalar.activation(
                        eS[:, half * 1280:(half + 1) * 1280], psX, AF.Exp,
                        scale=SM_SCALE)
                # diagonal-block masks: d0 via gpsimd into eSd (PV reads
                # eSd), d4 zeroed inside eS via a predicated write (DVE)
                for qt in range(NQT):
                    if half == 0:
                        c = _es_col(qt, qt)
                        for eS, eSd in ((eS_A, eSd_A), (eS_B, eSd_B)):
                            nc.gpsimd.tensor_mul(
                                eSd[:, qt, :], eS[:, c:c + P], tri_g)
                    else:
                        c = _es_col(qt, qt + 4)
                        for eS in (eS_A, eS_B):
                            nc.vector.copy_predicated(
                                eS[:, c:c + P], tri_p, zeros_sb)
                # previous pair's PV fills the PE while exp drains psA/psB
                if pending:
                    emit_pv(pending.pop(0), pending.pop(0), pending.pop(0))
            pending = [hA, eS_A, eSd_A, hB, eS_B, eSd_B]
        emit_pv(pending.pop(0), pending.pop(0), pending.pop(0))
        emit_pv(pending.pop(0), pending.pop(0), pending.pop(0))

    # ---- Stages D+E: attnT transpose + wo (fp8 DR) + residual + ffn norm ----
    attnT_stack = ExitStack()
    attnT_pool = attnT_stack.enter_context(tc.tile_pool(name="attnT_pool",
                                                        bufs=1))
    attnT = attnT_pool.tile([P, KD, OWN], FP8)

    stage_de = ExitStack()
    with stage_de:
        wo_pool = stage_de.enter_context(tc.tile_pool(name="wo_pool", bufs=1))
        wo_sb = wo_pool.tile([P, KD, D], FP8)
        nc.sync.dma_start(wo_sb, io["woT"].rearrange("(kd p) n -> p kd n", p=P))
        ps_r = stage_de.enter_context(
            tc.tile_pool(name="ps_r", bufs=2, space="PSUM"))
        tp_d = stage_de.enter_context(
            tc.tile_pool(name="tp_d", bufs=3, space="PSUM"))
        tp_ps3 = stage_de.enter_context(
            tc.tile_pool(name="tp_ps3", bufs=2, space="PSUM"))
        pe = stage_de.enter_context(tc.tile_pool(name="pe", bufs=2))

        def emit_attnT(qt):
            for kd in range(KD):
                pt = tp_d.tile([P, P], BF16, tag="tpd")
                nc.tensor.transpose(pt, attn_sb[:, qt, kd * P:(kd + 1) * P],
                                    identity)
                nc.vector.tensor_copy(attnT[:, kd, qt * P:(qt + 1) * P], pt)

        emit_attnT(0)
        emit_attnT(1)
        pend_hn = None
        for qt in range(NQT):
            xr = xown[:, qt, :]
            for half in range(2):
                ps = ps_r.tile([P, 512], F32, tag="psr")
                for j in range(KD // 2):
                    nc.tensor.matmul(
                        ps, lhsT=attnT[:, 2 * j:2 * j + 2, qt * P:(qt + 1) * P],
                        rhs=wo_sb[:, 2 * j:2 * j + 2,
                                  half * 512:(half + 1) * 512],
                        start=(j == 0), stop=(j == KD // 2 - 1), perf_mode=DR)
                nc.vector.scalar_tensor_tensor(
                    h_sb[:, qt, half * 512:(half + 1) * 512], ps, io["c_wo"],
                    xr[:, half * 512:(half + 1) * 512],
                    op0=mybir.AluOpType.mult, op1=mybir.AluOpType.add)
            if qt + 2 < NQT:
                emit_attnT(qt + 2)
            if pend_hn is not None:
                pqt, phn = pend_hn
                for kd in range(KD):
                    pt = tp_ps3.tile([P, P], BF16, tag="tp3")
                    nc.tensor.transpose(pt, phn[:, kd * P:(kd + 1) * P],
                                        identity)
                    nc.vector.tensor_copy(hnT[:, kd, pqt * P:(pqt + 1) * P], pt)
            # ffn rmsnorm; hn is scaled by 1/8 to undo the w1/w3 fp8 scale
            sqh = pe.tile([P, D], F32, tag="sqh")
            ssqh = sstat.tile([P, 1], F32, tag="ssq")
            nc.scalar.activation(sqh, h_sb[:, qt, :], AF.Square, accum_out=ssqh)
            stdh = sstat.tile([P, 1], F32, tag="std")
            nc.scalar.activation(stdh, ssqh, AF.Sqrt, bias=epsh_sb,
                                 scale=S_W13 * S_W13 / D)
            rstdh = sstat.tile([P, 1], F32, tag="rstd")
            nc.vector.reciprocal(rstdh, stdh)
            hn = pe.tile([P, D], BF16, tag="hn")
            nc.gpsimd.tensor_scalar_mul(hn, h_sb[:, qt, :], rstdh)
            pend_hn = (qt, hn)
        pqt, phn = pend_hn
        for kd in range(KD):
            pt = tp_ps3.tile([P, P], BF16, tag="tp3")
            nc.tensor.transpose(pt, phn[:, kd * P:(kd + 1) * P], identity)
            nc.vector.tensor_copy(hnT[:, kd, pqt * P:(pqt + 1) * P], pt)

    attnT_stack.close()
    xall_stack.close()
    ap_stack.close()

    # ---- Stage F: SwiGLU FFN (fp8 DoubleRow) ----
    stage_f = ExitStack()
    with stage_f:
        fT_pool = stage_f.enter_context(tc.tile_pool(name="fT_pool", bufs=1))
        fT = fT_pool.tile([P, KH, OWN], FP8)   # silu(g) * u, feature-major
        w13 = stage_f.enter_context(tc.tile_pool(name="w13", bufs=4))
        ps_f = stage_f.enter_context(
            tc.tile_pool(name="ps_f", bufs=2, space="PSUM"))
        pf = stage_f.enter_context(tc.tile_pool(name="pf", bufs=2))

        for mi in range(KH):
            w1t = w13.tile([P, KD, P], FP8, tag="w1t")
            nc.sync.dma_start(
                w1t, io["w1T"][:, mi * P:(mi + 1) * P]
                .rearrange("(kd p) m -> p kd m", p=P))
            w3t = w13.tile([P, KD, P], FP8, tag="w3t")
            nc.sync.dma_start(
                w3t, io["w3T"][:, mi * P:(mi + 1) * P]
                .rearrange("(kd p) m -> p kd m", p=P))
            psg = ps_f.tile([P, 512], F32, tag="pg")
            for j in range(KD // 2):
                nc.tensor.matmul(psg, lhsT=w1t[:, 2 * j:2 * j + 2, :],
                                 rhs=hnT[:, 2 * j:2 * j + 2, :],
                                 start=(j == 0), stop=(j == KD // 2 - 1),
                                 perf_mode=DR)
            psu = ps_f.tile([P, 512], F32, tag="pu")
            for j in range(KD // 2):
                nc.tensor.matmul(psu, lhsT=w3t[:, 2 * j:2 * j + 2, :],
                                 rhs=hnT[:, 2 * j:2 * j + 2, :],
                                 start=(j == 0), stop=(j == KD // 2 - 1),
                                 perf_mode=DR)
            # psg/psu are exact g/u (scales cancelled): silu via sigmoid
            sg = pf.tile([P, 512], F32, tag="sg")
            nc.scalar.activation(sg, psg, AF.Sigmoid)
            gm = pf.tile([P, 512], F32, tag="gm")
            nc.vector.tensor_mul(gm, sg, psg)
            nc.vector.tensor_mul(fT[:, mi, :], gm, psu)

        w2_pool = stage_f.enter_context(tc.tile_pool(name="w2_pool", bufs=1))
        w2_sb = w2_pool.tile([P, KH, D], FP8)
        for c in range(8):
            nc.sync.dma_start(
                w2_sb[:, c * 4:(c + 1) * 4, :],
                io["w2T"][c * 4 * P:(c + 1) * 4 * P, :]
                .rearrange("(kh p) n -> p kh n", p=P))
        ps_y = stage_f.enter_context(
            tc.tile_pool(name="ps_y", bufs=2, space="PSUM"))
        py = stage_f.enter_context(tc.tile_pool(name="py", bufs=2))

        for qt in range(NQT):
            yt = py.tile([P, D], F32, tag="yt")
            for half in range(2):
                ps = ps_y.tile([P, 512], F32, tag="psy")
                for j in range(KH // 2):
                    nc.tensor.matmul(
                        ps, lhsT=fT[:, 2 * j:2 * j + 2, qt * P:(qt + 1) * P],
                        rhs=w2_sb[:, 2 * j:2 * j + 2,
                                  half * 512:(half + 1) * 512],
                        start=(j == 0), stop=(j == KH // 2 - 1), perf_mode=DR)
                # undo the w2 fp8 range scale and apply ffn_scale's scalar
                nc.vector.scalar_tensor_tensor(
                    yt[:, half * 512:(half + 1) * 512], ps, io["c_y"],
                    h_sb[:, qt, half * 512:(half + 1) * 512],
                    op0=mybir.AluOpType.mult, op1=mybir.AluOpType.add)
            nc.sync.dma_start(y[qt * P:(qt + 1) * P, :], yt)


def build_nc(c_y: float, c_wo: float):
    nc = bacc.Bacc("TRN2", target_bir_lowering=False, debug=False,
                   num_devices=NCORES)
    io = {
        "xbf": nc.dram_tensor("xbf", [CTX, D], BF16, kind="ExternalInput").ap(),
        "xTf8": nc.dram_tensor("xTf8", [D, CTX], FP8,
                               kind="ExternalInput").ap(),
        "xown": nc.dram_tensor("xown", [OWN, D], F32,
                               kind="ExternalInput").ap(),
        "wqT": nc.dram_tensor("wqT", [D, D], FP8, kind="ExternalInput").ap(),
        "wkvT": nc.dram_tensor("wkvT", [D, 2 * KVH * HD], FP8,
                               kind="ExternalInput").ap(),
        "woT": nc.dram_tensor("woT", [D, D], FP8, kind="ExternalInput").ap(),
        "w1T": nc.dram_tensor("w1T", [D, HID], FP8, kind="ExternalInput").ap(),
        "w3T": nc.dram_tensor("w3T", [D, HID], FP8, kind="ExternalInput").ap(),
        "w2T": nc.dram_tensor("w2T", [HID, D], FP8, kind="ExternalInput").ap(),
        "qw2": nc.dram_tensor("qw2", [P, 1], F32, kind="ExternalInput").ap(),
        "kw2": nc.dram_tensor("kw2", [P, 1], F32, kind="ExternalInput").ap(),
        "tri_g": nc.dram_tensor("tri_g", [P, P], BF16,
                                kind="ExternalInput").ap(),
        "tri_p": nc.dram_tensor("tri_p", [P, P], mybir.dt.uint8,
                                kind="ExternalInput").ap(),
        "vones": nc.dram_tensor("vones", [P, NKT], BF16,
                                kind="ExternalInput").ap(),
        "y": nc.dram_tensor("y", [OWN, D], F32, kind="ExternalOutput").ap(),
        "c_y": c_y,
        "c_wo": c_wo,
    }
    with tile.TileContext(nc) as tc:
        with ExitStack() as ctx:
            _build_tile_kernel(ctx, tc, io)
    nc.compile()
    return nc


_CACHE = {}


def get_nc(c_y: float, c_wo: float):
    if "nc" not in _CACHE:
        _CACHE["nc"] = build_nc(c_y, c_wo)
    return _CACHE["nc"]


def _fp8(a):
    return np.ascontiguousarray(
        np.clip(a, -240.0, 240.0)).astype(NPFP8)


def prep_in_maps(inputs):
    """Fold scales into weights, transpose/cast, and slice per-core inputs."""
    f32 = np.float32
    x = np.asarray(inputs["x"], f32)
    wq = np.asarray(inputs["wq"], f32)
    wk = np.asarray(inputs["wk"], f32)
    wv = np.asarray(inputs["wv"], f32)
    wo = np.asarray(inputs["wo"], f32)
    w1 = np.asarray(inputs["w1"], f32)
    w2 = np.asarray(inputs["w2"], f32)
    w3 = np.asarray(inputs["w3"], f32)
    qw = np.asarray(inputs["q_norm_w"], f32)
    kw = np.asarray(inputs["k_norm_w"], f32)
    anw = np.asarray(inputs["attn_norm_w"], f32)
    fnw = np.asarray(inputs["ffn_norm_w"], f32)
    asc = np.asarray(inputs["attn_scale"], f32)
    fsc = np.asarray(inputs["ffn_scale"], f32)

    HEAD_PERM = [0, 4, 1, 5, 2, 6, 3, 7, 8, 12, 9, 13, 10, 14, 11, 15]
    wq_p = (wq * anw[None, :]).reshape(H, HD, D)[HEAD_PERM].reshape(H * HD, D)
    wqT = _fp8(wq_p.T * S_WQKV)
    wkvT = _fp8(
        np.concatenate([wk * anw[None, :], wv * anw[None, :]], axis=0).T
        * S_WQKV)
    asc_s = float(np.mean(asc))
    c_wo = asc_s / S_WO
    wo_p = ((wo * (asc / np.float32(asc_s))[:, None])
            .T.reshape(H, HD, D)[HEAD_PERM].reshape(H * HD, D))
    woT = _fp8(wo_p * S_WO)
    w1T = _fp8((w1 * fnw[None, :]).T * S_W13)
    w3T = _fp8((w3 * fnw[None, :]).T * S_W13)
    fsc_s = float(np.mean(fsc))
    c_y = fsc_s / S_W2
    w2T = _fp8((w2 * (fsc / np.float32(fsc_s))[:, None]).T * S_W2)
    qwb = np.ascontiguousarray(np.tile(qw, 2)[:, None]).astype(f32)
    kwb = np.ascontiguousarray(np.tile(kw, 2)[:, None]).astype(f32)

    # diagonal-block triangle masks:
    # d0 block (ki==qt): valid iff k > qq (bf16 VALID mask, gpsimd multiply)
    # d4 block (ki==qt+4): valid iff k <= qq (uint8 INVALID mask, DVE zeroing)
    k_i = np.arange(P)[:, None]
    q_i = np.arange(P)[None, :]
    tri_g = np.ascontiguousarray((k_i > q_i).astype(NPBF16))
    tri_p = np.ascontiguousarray((k_i > q_i).astype(np.uint8))

    # per-token validity for V's appended column (0 for halo padding)
    v_int = np.ones((P, NKT), NPBF16)
    v_first = np.zeros((P, NKT), NPBF16)
    v_first[:, NQT:] = 1.0

    shared = dict(wqT=wqT, wkvT=wkvT, woT=woT, w1T=w1T, w3T=w3T, w2T=w2T,
                  qw2=qwb, kw2=kwb, tri_g=tri_g, tri_p=tri_p)
    in_maps = []
    for b in range(B):
        for j in range(T // OWN):
            xc = np.zeros((CTX, D), f32)
            if j == 0:
                xc[OWN:] = x[b, 0:OWN]
                vm = v_first
            else:
                xc[:] = x[b, (j - 1) * OWN:(j + 1) * OWN]
                vm = v_int
            in_maps.append(dict(xbf=xc.astype(NPBF16),
                                xTf8=_fp8(np.ascontiguousarray(xc.T)),
                                xown=np.ascontiguousarray(xc[OWN:]),
                                vones=vm, **shared))
    return in_maps, c_y, c_wo


LAST_RESULTS = None


def _ensure_ntff_hook():
    """Install the axon NTFF profile hook if the image's antenv lacks it."""
    import types
    try:
        from antenv.axon_hooks import get_axon_ntff_profile_hook  # noqa: F401
        return  # real module present
    except ImportError:
        pass
    try:
        import antenv
        boot_dir = "/root/.axon_site/trn_agent_boot"
        if boot_dir not in sys.path:
            sys.path.insert(0, boot_dir)
        import trn_boot
        hook = trn_boot._ntff_profile_via_ctypes("/opt/axon/libaxon_pjrt.so")
        mod = types.ModuleType("antenv.axon_hooks")
        mod._hook = hook
        mod.get_axon_ntff_profile_hook = lambda: mod._hook
        mod.set_axon_ntff_profile_hook = lambda h: setattr(mod, "_hook", h)
        sys.modules["antenv.axon_hooks"] = mod
        antenv.axon_hooks = mod
        import concourse.bass_utils as _bu
        _bu.upload_artifacts = lambda tmpdir: tmpdir
    except Exception as e:  # pragma: no cover
        print(f"ntff hook unavailable ({e}); running without trace")


def kernel(**inputs):
    global LAST_RESULTS
    if os.environ.get("BASS_TRACE"):
        _ensure_ntff_hook()
    in_maps, c_y, c_wo = prep_in_maps(inputs)
    nc = get_nc(c_y, c_wo)
    res = run_bass_kernel_spmd(nc, in_maps, core_ids=list(range(NCORES)))
    LAST_RESULTS = res
    y = np.empty((B, T, D), np.float32)
    for c in range(NCORES):
        b, j = divmod(c, T // OWN)
        y[b, j * OWN:(j + 1) * OWN] = res.results[c]["y"]
    return y


# revision 9
# speedup vs baseline: 1.2229x; 1.2229x over previous
"""Trainium2 Bass kernel for a codec-transformer block (sliding-window GQA + SwiGLU).

Sharding: data-parallel over 8 token chunks (2 batches x 4 chunks of 512
tokens). The 512-token sliding window makes attention local: each core
receives its 512 "own" tokens plus the preceding 512 tokens as a KV halo,
so no collectives are needed.

Host-side prep (layout only, no model FLOPs):
  - attn_norm_w folded into wq/wk/wv columns, ffn_norm_w into w1/w3 columns
  - attn_scale/ffn_scale: their scalar parts are applied on-chip (c_wo, c_y)
    so the fp8 weights keep a healthy range; only the shape (ratio to the
    scalar) is folded into wo/w2 rows
  - wq/wkv/wo/w1/w3/w2 are cast to fp8e4 with power-of-two range scales;
    every scale is absorbed into an op the kernel already needs:
      * wq,wk x32: cancels in qk-rmsnorm (scale-invariant)
      * wv x32: undone by the per-token rstd fold at the V eviction
      * w1,w3 x8: undone by scaling hn by 1/8 (folded into the hn-rmsnorm
        sqrt scale), making psg/psu exact so fT = silu(psg)*psu is exact
      * wo x16 / w2 x16: undone by the c_wo/c_y constants on the h/y paths
  - x is sent twice: bf16 [CTX,D] for the transpose/matmul path and f32
    [OWN,D] for the residual; the x-rmsnorm cancels in qk-norm for Q/K and
    is applied to V via rstd at its eviction, so raw x feeds the PE
    transposes directly (no norm on the critical path).

Attention stage: head pairs (sharing a kv pair-transposed kT2 tile) run
concurrently on PE row groups 0-63/64-127. Scores for one head live in two
[P,1280] PSUM tiles with a ragged ki-permuted layout chosen so every
matmul output stays inside a 2KB bank; exp then runs as ONE activation per
half-head. The sliding-window mask reduces to two constant 128x128
triangles applied only to the two diagonal blocks per query tile
(copy_predicated with zeros); halo-padding tokens are excluded via a
0/1 validity column in V's appended ones-column, which zeroes both their
numerator and softmax-denominator contributions.
"""

import os
import sys

sys.path.insert(0, "/opt/trn_rl_repo")
os.environ.setdefault("MYCRO_LOCAL_CACHE", "1")

from contextlib import ExitStack

import numpy as np
import ml_dtypes

import concourse.bass as bass
import concourse.bacc as bacc
import concourse.tile as tile
from concourse import mybir
from concourse.masks import make_identity
from concourse.bass_utils import run_bass_kernel_spmd

BF16 = mybir.dt.bfloat16
F32 = mybir.dt.float32
FP8 = mybir.dt.float8e4
AF = mybir.ActivationFunctionType
DR = mybir.MatmulPerfMode.DoubleRow
NPBF16 = ml_dtypes.bfloat16
NPFP8 = ml_dtypes.float8_e4m3

P = 128
B, T, D = 2, 2048, 1024
HID = 4096
H, KVH, HD = 16, 4, 64
KD = D // P            # 8 contraction tiles over model dim
KH = HID // P          # 32 contraction tiles over hidden dim
OWN = 512              # tokens owned per core
CTX = 1024             # own + 512-token halo
NQT = OWN // P         # 4
NKT = CTX // P         # 8
NCORES = 8
KC = KVH * HD          # 256
EPS = 1e-5
QKEPS = 1e-6
SM_SCALE = 1.0 / 8.0   # 1/sqrt(HD)

S_WQKV = 32.0          # fp8 range scale on wq/wk/wv
S_W13 = 8.0            # fp8 range scale on w1/w3 (alpha = 1/8 on hn)
S_W2 = 16.0            # fp8 range scale on w2
S_WO = 16.0            # fp8 range scale on wo

# Ragged in-bank PSUM layout for one half-head of scores ([P,1280] f32).
# Widths per ki: 128,256,384,512,512,384,256,128; this permutation keeps
# every matmul output inside a 2KB (512-f32) PSUM bank.
OFF = {0: 896, 1: 1024, 2: 512, 3: 0,
       4: 1280 + 0, 5: 1280 + 512, 6: 1280 + 1024, 7: 1280 + 896}


def _qclip(ki):
    """Valid own-query range for ctx key tile ki under the sliding window."""
    return max(0, P * (ki - 4)), min(OWN, P * (ki + 1))


def _es_col(qt, ki):
    """eS/psum column of query-tile block (qt, ki) in the ragged layout."""
    return OFF[ki] + qt * P - _qclip(ki)[0]


def _build_tile_kernel(ctx: ExitStack, tc: tile.TileContext, io: dict):
    nc = tc.nc
    y = io["y"]

    const = ctx.enter_context(tc.tile_pool(name="const", bufs=1))
    identity = const.tile([P, P], BF16)
    make_identity(nc, identity)
    qw2_sb = const.tile([P, 1], F32)    # q_norm_w tiled over both 64-rows
    nc.sync.dma_start(qw2_sb, io["qw2"])
    kw2_sb = const.tile([P, 1], F32)
    nc.sync.dma_start(kw2_sb, io["kw2"])
    tri_g = const.tile([P, P], BF16)        # d0 VALID mask (k > qq), gpsimd
    nc.sync.dma_start(tri_g, io["tri_g"])
    tri_p = const.tile([P, P], mybir.dt.uint8)  # d4 INVALID mask (k > qq), DVE
    nc.sync.dma_start(tri_p, io["tri_p"])
    zeros_sb = const.tile([P, P], BF16)
    nc.vector.memset(zeros_sb, 0.0)
    vm_sb = const.tile([P, NKT], BF16)      # per-token validity (halo pad=0)
    nc.sync.dma_start(vm_sb, io["vones"])
    epsv_sb = const.tile([P, 1], F32)
    nc.vector.memset(epsv_sb, EPS * S_WQKV * S_WQKV)
    epsh_sb = const.tile([P, 1], F32)
    nc.vector.memset(epsh_sb, EPS * S_W13 * S_W13)
    qkeps_sb = const.tile([P, 1], F32)
    nc.vector.memset(qkeps_sb, QKEPS)

    sstat = ctx.enter_context(tc.tile_pool(name="sstat", bufs=8))

    pers = ctx.enter_context(tc.tile_pool(name="pers", bufs=1))
    h_sb = pers.tile([P, NQT, D], F32)       # residual h = x + r, fp32
    hnT_pool = ctx.enter_context(tc.tile_pool(name="hnT_pool", bufs=1))
    hnT = hnT_pool.tile([P, KD, OWN], FP8)
    wqkv_pool = ctx.enter_context(tc.tile_pool(name="wqkv", bufs=1))
    wkv_sb = wqkv_pool.tile([P, KD, 2 * KC], FP8)
    nc.sync.dma_start(wkv_sb, io["wkvT"].rearrange("(kd p) n -> p kd n", p=P))
    wq_sb = wqkv_pool.tile([P, KD, D], FP8)
    nc.sync.dma_start(wq_sb, io["wqT"].rearrange("(kd p) n -> p kd n", p=P))
    ap_stack = ExitStack()
    attn_pers = ap_stack.enter_context(tc.tile_pool(name="attn_pers", bufs=1))
    # qhat^T: q heads are laid out (via the host-side wq column permutation)
    # so head h lives in feature tile tau=(h%4)+4*(h//8) at partition base
    # pi=((h//4)%2)*64 -- exactly where its kv head lands in kT2's natural
    # pair-transpose layout, so scores operands always share a base partition.
    qkT = attn_pers.tile([P, KD, OWN], BF16)
    kT2 = attn_pers.tile([P, 2, CTX], BF16)
    v65 = attn_pers.tile([P, NKT, KVH, HD + 1], BF16)  # v tokens + valid col
    attn_sb = attn_pers.tile([P, NQT, H * HD], BF16)  # attn out, token-major
    for kvh in range(KVH):
        nc.vector.tensor_copy(v65[:, :, kvh, HD:HD + 1], vm_sb[:, :, None])

    xall_stack = ExitStack()
    xall_pool = xall_stack.enter_context(tc.tile_pool(name="xall", bufs=1))
    xbf = xall_pool.tile([P, NKT, D], BF16)
    xown = xall_pool.tile([P, NQT, D], F32)
    xT = xall_pool.tile([P, KD, CTX], FP8)
    for kd in range(KD):
        nc.sync.dma_start(xT[:, kd, :],
                          io["xTf8"][kd * P:(kd + 1) * P, :])
    for i in range(NKT):
        nc.sync.dma_start(xbf[:, i, :], io["xbf"][i * P:(i + 1) * P, :])
    for i in range(NQT):
        nc.sync.dma_start(xown[:, i, :], io["xown"][i * P:(i + 1) * P, :])

    # ---- Stages A+B: x transpose + QKV (fp8 DoubleRow), per ctx tile ----
    stage_a = ExitStack()
    with stage_a:
        tp_ps = stage_a.enter_context(
            tc.tile_pool(name="tp_ps", bufs=3, space="PSUM"))
        pb_ps = stage_a.enter_context(
            tc.tile_pool(name="pb_ps", bufs=5, space="PSUM"))
        pb = stage_a.enter_context(tc.tile_pool(name="pb", bufs=3))

        def stats_tile(i):
            """sum(x^2) -> rstd_v = 1/(32*sqrt(mean+eps)) (ACT/DVE only)."""
            sq = pb.tile([P, D], F32, tag="sq")
            ssq = sstat.tile([P, 1], F32, tag="ssq")
            nc.scalar.activation(sq, xbf[:, i, :], AF.Square, accum_out=ssq)
            stdv = sstat.tile([P, 1], F32, tag="std")
            nc.scalar.activation(stdv, ssq, AF.Sqrt, bias=epsv_sb,
                                 scale=S_WQKV * S_WQKV / D)
            rstd_v = sstat.tile([P, 1], F32, tag="rstdv")
            nc.vector.reciprocal(rstd_v, stdv)
            return rstd_v

        def emit_k_tp(kt, khat):
            # eviction applies k_norm_w (per feature = per partition here)
            pt = tp_ps.tile([P, 2, P], BF16, tag="tp")
            for kf in range(2):
                nc.tensor.transpose(pt[:, kf, :],
                                    khat[:, kf * P:(kf + 1) * P], identity)
            nc.vector.tensor_scalar_mul(
                kT2[:, :, kt * P:(kt + 1) * P], pt, kw2_sb)

        def emit_q_tp(qt, qhats):
            for half in range(2):
                for j in range(0, 4, 2):
                    pt = tp_ps.tile([P, 2, P], BF16, tag="tp")
                    nc.tensor.transpose(
                        pt[:, 0, :], qhats[half][:, j * P:(j + 1) * P],
                        identity)
                    nc.tensor.transpose(
                        pt[:, 1, :], qhats[half][:, (j + 1) * P:(j + 2) * P],
                        identity)
                    nc.vector.tensor_scalar_mul(
                        qkT[:, half * 4 + j:half * 4 + j + 2,
                            qt * P:(qt + 1) * P], pt, qw2_sb)

        # kq-hat transposes run TWO tiles behind their matmuls so the
        # qk-norm ACT/DVE chains never stall the PE stream.
        rstds = {0: stats_tile(0), 1: stats_tile(1)}
        pend_k = {}
        pend_q = {}
        for i in range(NKT):
            rstd_v = rstds.pop(i)
            # K / V projection for ctx tile i (fp8 DoubleRow over kd pairs)
            ps = pb_ps.tile([P, 512], F32, tag="ps")
            for j in range(KD // 2):
                nc.tensor.matmul(
                    ps, lhsT=xT[:, 2 * j:2 * j + 2, i * P:(i + 1) * P],
                    rhs=wkv_sb[:, 2 * j:2 * j + 2, :],
                    start=(j == 0), stop=(j == KD // 2 - 1), perf_mode=DR)
            kv_ps = ps

            # Q projection for own tile qt = i - 4
            q_pss = None
            if i >= NQT:
                qt = i - NQT
                col = OWN + qt * P
                q_pss = []
                for half in range(2):
                    ps = pb_ps.tile([P, 512], F32, tag="ps")
                    q_pss.append(ps)
                    for j in range(KD // 2):
                        nc.tensor.matmul(
                            ps, lhsT=xT[:, 2 * j:2 * j + 2, col:col + P],
                            rhs=wq_sb[:, 2 * j:2 * j + 2,
                                      half * 512:(half + 1) * 512],
                            start=(j == 0), stop=(j == KD // 2 - 1),
                            perf_mode=DR)

            # two-behind transposes keep the PE stream dense
            if i - 2 in pend_k:
                emit_k_tp(i - 2, pend_k.pop(i - 2))
            if i - 2 - NQT in pend_q:
                emit_q_tp(i - 2 - NQT, pend_q.pop(i - 2 - NQT))

            # k-chain + v eviction (ACT/DVE)
            ps = kv_ps
            sqk = pb.tile([P, KC], F32, tag="sqk")
            nc.scalar.activation(sqk, ps[:, 0:KC], AF.Square)
            msk = pb.tile([P, KVH], F32, tag="msk")
            nc.vector.reduce_sum(
                msk, sqk.rearrange("p (h e) -> p h e", e=HD),
                axis=mybir.AxisListType.X)
            sck = sstat.tile([P, KVH], F32, tag="sck")
            nc.scalar.activation(sck, msk, AF.Sqrt, bias=qkeps_sb, scale=1.0 / HD)
            rck = sstat.tile([P, KVH], F32, tag="rck")
            nc.vector.reciprocal(rck, sck)
            khat = pb.tile([P, KC], BF16, tag="khat")
            nc.vector.tensor_mul(
                khat.rearrange("p (h e) -> p h e", e=HD),
                ps[:, 0:KC].rearrange("p (h e) -> p h e", e=HD),
                rck[:, :, None].broadcast_to([P, KVH, HD]))
            pend_k[i] = khat
            nc.vector.tensor_scalar_mul(
                v65[:, i, :, 0:HD],
                ps[:, KC:2 * KC].rearrange("p (h e) -> p h e", e=HD),
                rstd_v)

            # q-chain
            if q_pss is not None:
                qt = i - NQT
                msq = pb.tile([P, H], F32, tag="msq")
                for half in range(2):
                    sqq = pb.tile([P, 512], F32, tag="sqq")
                    nc.scalar.activation(sqq, q_pss[half], AF.Square)
                    nc.vector.reduce_sum(
                        msq[:, half * 8:(half + 1) * 8],
                        sqq.rearrange("p (h e) -> p h e", e=HD),
                        axis=mybir.AxisListType.X)
                sc = sstat.tile([P, H], F32, tag="sc")
                nc.scalar.activation(sc, msq, AF.Sqrt, bias=qkeps_sb,
                                     scale=1.0 / HD)
                rc = sstat.tile([P, H], F32, tag="rc")
                nc.vector.reciprocal(rc, sc)
                qhats = []
                for half in range(2):
                    ps = q_pss[half]
                    qhat = pb.tile([P, 512], BF16, tag="qhat")
                    nc.vector.tensor_mul(
                        qhat.rearrange("p (h e) -> p h e", e=HD),
                        ps.rearrange("p (h e) -> p h e", e=HD),
                        rc[:, half * 8:(half + 1) * 8, None]
                        .broadcast_to([P, 8, HD]))
                    qhats.append(qhat)
                pend_q[qt] = qhats

            if i + 2 < NKT:
                rstds[i + 2] = stats_tile(i + 2)

        for i in (NKT - 2, NKT - 1):
            emit_k_tp(i, pend_k.pop(i))
        for qt in (NQT - 2, NQT - 1):
            emit_q_tp(qt, pend_q.pop(qt))

    # ---- Stage C: attention. Head pairs run on PE row groups 0/64. ----
    stage_c = ExitStack()
    with stage_c:
        es_pool = stage_c.enter_context(tc.tile_pool(name="es_pool", bufs=2))
        psc = stage_c.enter_context(
            tc.tile_pool(name="psc", bufs=1, space="PSUM"))
        ps_o = stage_c.enter_context(
            tc.tile_pool(name="ps_o", bufs=2, space="PSUM"))

        def emit_pv(h, eS, eSd):
            kvh = h // 4
            tau = (h % 4) + 4 * (h // 8)
            pi = ((h // 4) % 2)
            slot = 2 * tau + pi
            for qt in range(NQT):
                po = ps_o.tile([P, HD + 1], F32, tag="po")
                for j in range(5):
                    if j == 0:
                        lhs = eSd[:, qt, :]
                    else:
                        c = _es_col(qt, qt + j)
                        lhs = eS[:, c:c + P]
                    nc.tensor.matmul(
                        po, lhsT=lhs,
                        rhs=v65[:, qt + j, kvh, :],
                        start=(j == 0), stop=(j == 4))
                rec = sstat.tile([P, 1], F32, tag="rec")
                nc.vector.reciprocal(rec, po[:, HD:HD + 1])
                nc.vector.tensor_scalar_mul(
                    attn_sb[:, qt, slot * HD:(slot + 1) * HD], po[:, 0:HD],
                    rec)

        PAIRS = [(0, 4), (1, 5), (2, 6), (3, 7),
                 (8, 12), (9, 13), (10, 14), (11, 15)]
        pending = []
        for hA, hB in PAIRS:
            g = hA // 8
            tau = (hA % 4) + 4 * (hA // 8)
            eS_A = es_pool.tile([P, 2 * 1280], BF16, tag="esA")
            eS_B = es_pool.tile([P, 2 * 1280], BF16, tag="esB")
            eSd_A = es_pool.tile([P, NQT, P], BF16, tag="esdA")
            eSd_B = es_pool.tile([P, NQT, P], BF16, tag="esdB")
            for half in range(2):
                psA = psc.tile([P, 1280], F32, tag="psA")
                psB = psc.tile([P, 1280], F32, tag="psB")
                for ki in range(half * 4, half * 4 + 4):
                    qlo, qhi = _qclip(ki)
                    w = qhi - qlo
                    o = OFF[ki] - half * 1280
                    nc.tensor.matmul(
                        psA[:, o:o + w],
                        lhsT=kT2[0:HD, g, ki * P:(ki + 1) * P],
                        rhs=qkT[0:HD, tau, qlo:qhi],
                        start=True, stop=True)
                    nc.tensor.matmul(
                        psB[:, o:o + w],
                        lhsT=kT2[HD:P, g, ki * P:(ki + 1) * P],
                        rhs=qkT[HD:P, tau, qlo:qhi],
                        start=True, stop=True)
                for eS, psX in ((eS_A, psA), (eS_B, psB)):
                    nc.scalar.activation(
                        eS[:, half * 1280:(half + 1) * 1280], psX, AF.Exp,
                        scale=SM_SCALE)
                # diagonal-block masks: d0 via gpsimd into eSd (PV reads
                # eSd), d4 zeroed inside eS via a predicated write (DVE)
                for qt in range(NQT):
                    if half == 0:
                        c = _es_col(qt, qt)
                        for eS, eSd in ((eS_A, eSd_A), (eS_B, eSd_B)):
                            nc.gpsimd.tensor_mul(
                                eSd[:, qt, :], eS[:, c:c + P], tri_g)
                    else:
                        c = _es_col(qt, qt + 4)
                        for eS in (eS_A, eS_B):
                            nc.vector.copy_predicated(
                                eS[:, c:c + P], tri_p, zeros_sb)
                # previous pair's PV fills the PE while exp drains psA/psB
                if pending:
                    emit_pv(pending.pop(0), pending.pop(0), pending.pop(0))
            pending = [hA, eS_A, eSd_A, hB, eS_B, eSd_B]
        emit_pv(pending.pop(0), pending.pop(0), pending.pop(0))
        emit_pv(pending.pop(0), pending.pop(0), pending.pop(0))

    # ---- Stages D+E: attnT transpose + wo (fp8 DR) + residual + ffn norm ----
    attnT_stack = ExitStack()
    attnT_pool = attnT_stack.enter_context(tc.tile_pool(name="attnT_pool",
                                                        bufs=1))
    attnT = attnT_pool.tile([P, KD, OWN], FP8)

    stage_de = ExitStack()
    with stage_de:
        wo_pool = stage_de.enter_context(tc.tile_pool(name="wo_pool", bufs=1))
        wo_sb = wo_pool.tile([P, KD, D], FP8)
        nc.sync.dma_start(wo_sb, io["woT"].rearrange("(kd p) n -> p kd n", p=P))
        ps_r = stage_de.enter_context(
            tc.tile_pool(name="ps_r", bufs=2, space="PSUM"))
        tp_d = stage_de.enter_context(
            tc.tile_pool(name="tp_d", bufs=3, space="PSUM"))
        tp_ps3 = stage_de.enter_context(
            tc.tile_pool(name="tp_ps3", bufs=2, space="PSUM"))
        pe = stage_de.enter_context(tc.tile_pool(name="pe", bufs=2))

        def emit_attnT(qt):
            for kd in range(KD):
                pt = tp_d.tile([P, P], BF16, tag="tpd")
                nc.tensor.transpose(pt, attn_sb[:, qt, kd * P:(kd + 1) * P],
                                    identity)
                nc.vector.tensor_copy(attnT[:, kd, qt * P:(qt + 1) * P], pt)

        emit_attnT(0)
        emit_attnT(1)
        pend_hn = None
        for qt in range(NQT):
            xr = xown[:, qt, :]
            for half in range(2):
                ps = ps_r.tile([P, 512], F32, tag="psr")
                for j in range(KD // 2):
                    nc.tensor.matmul(
                        ps, lhsT=attnT[:, 2 * j:2 * j + 2, qt * P:(qt + 1) * P],
                        rhs=wo_sb[:, 2 * j:2 * j + 2,
                                  half * 512:(half + 1) * 512],
                        start=(j == 0), stop=(j == KD // 2 - 1), perf_mode=DR)
                nc.vector.scalar_tensor_tensor(
                    h_sb[:, qt, half * 512:(half + 1) * 512], ps, io["c_wo"],
                    xr[:, half * 512:(half + 1) * 512],
                    op0=mybir.AluOpType.mult, op1=mybir.AluOpType.add)
            if qt + 2 < NQT:
                emit_attnT(qt + 2)
            if pend_hn is not None:
                pqt, phn = pend_hn
                for kd in range(KD):
                    pt = tp_ps3.tile([P, P], BF16, tag="tp3")
                    nc.tensor.transpose(pt, phn[:, kd * P:(kd + 1) * P],
                                        identity)
                    nc.vector.tensor_copy(hnT[:, kd, pqt * P:(pqt + 1) * P], pt)
            # ffn rmsnorm; hn is scaled by 1/8 to undo the w1/w3 fp8 scale
            sqh = pe.tile([P, D], F32, tag="sqh")
            ssqh = sstat.tile([P, 1], F32, tag="ssq")
            nc.scalar.activation(sqh, h_sb[:, qt, :], AF.Square, accum_out=ssqh)
            stdh = sstat.tile([P, 1], F32, tag="std")
            nc.scalar.activation(stdh, ssqh, AF.Sqrt, bias=epsh_sb,
                                 scale=S_W13 * S_W13 / D)
            rstdh = sstat.tile([P, 1], F32, tag="rstd")
            nc.vector.reciprocal(rstdh, stdh)
            hn = pe.tile([P, D], BF16, tag="hn")
            nc.vector.tensor_scalar_mul(hn, h_sb[:, qt, :], rstdh)
            pend_hn = (qt, hn)
        pqt, phn = pend_hn
        for kd in range(KD):
            pt = tp_ps3.tile([P, P], BF16, tag="tp3")
            nc.tensor.transpose(pt, phn[:, kd * P:(kd + 1) * P], identity)
            nc.vector.tensor_copy(hnT[:, kd, pqt * P:(pqt + 1) * P], pt)

    attnT_stack.close()
    xall_stack.close()
    ap_stack.close()

    # ---- Stage F: SwiGLU FFN (fp8 DoubleRow) ----
    stage_f = ExitStack()
    with stage_f:
        fT_pool = stage_f.enter_context(tc.tile_pool(name="fT_pool", bufs=1))
        fT = fT_pool.tile([P, KH, OWN], FP8)   # silu(g) * u, feature-major
        w13 = stage_f.enter_context(tc.tile_pool(name="w13", bufs=4))
        ps_f = stage_f.enter_context(
            tc.tile_pool(name="ps_f", bufs=2, space="PSUM"))
        pf = stage_f.enter_context(tc.tile_pool(name="pf", bufs=2))

        for mi in range(KH):
            w1t = w13.tile([P, KD, P], FP8, tag="w1t")
            nc.sync.dma_start(
                w1t, io["w1T"][:, mi * P:(mi + 1) * P]
                .rearrange("(kd p) m -> p kd m", p=P))
            w3t = w13.tile([P, KD, P], FP8, tag="w3t")
            nc.sync.dma_start(
                w3t, io["w3T"][:, mi * P:(mi + 1) * P]
                .rearrange("(kd p) m -> p kd m", p=P))
            psg = ps_f.tile([P, 512], F32, tag="pg")
            for j in range(KD // 2):
                nc.tensor.matmul(psg, lhsT=w1t[:, 2 * j:2 * j + 2, :],
                                 rhs=hnT[:, 2 * j:2 * j + 2, :],
                                 start=(j == 0), stop=(j == KD // 2 - 1),
                                 perf_mode=DR)
            psu = ps_f.tile([P, 512], F32, tag="pu")
            for j in range(KD // 2):
                nc.tensor.matmul(psu, lhsT=w3t[:, 2 * j:2 * j + 2, :],
                                 rhs=hnT[:, 2 * j:2 * j + 2, :],
                                 start=(j == 0), stop=(j == KD // 2 - 1),
                                 perf_mode=DR)
            # psg/psu are exact g/u (scales cancelled): silu via sigmoid
            sg = pf.tile([P, 512], F32, tag="sg")
            nc.scalar.activation(sg, psg, AF.Sigmoid)
            gm = pf.tile([P, 512], F32, tag="gm")
            nc.vector.tensor_mul(gm, sg, psg)
            nc.vector.tensor_mul(fT[:, mi, :], gm, psu)

        w2_pool = stage_f.enter_context(tc.tile_pool(name="w2_pool", bufs=1))
        w2_sb = w2_pool.tile([P, KH, D], FP8)
        for c in range(8):
            nc.sync.dma_start(
                w2_sb[:, c * 4:(c + 1) * 4, :],
                io["w2T"][c * 4 * P:(c + 1) * 4 * P, :]
                .rearrange("(kh p) n -> p kh n", p=P))
        ps_y = stage_f.enter_context(
            tc.tile_pool(name="ps_y", bufs=2, space="PSUM"))
        py = stage_f.enter_context(tc.tile_pool(name="py", bufs=2))

        for qt in range(NQT):
            yt = py.tile([P, D], F32, tag="yt")
            for half in range(2):
                ps = ps_y.tile([P, 512], F32, tag="psy")
                for j in range(KH // 2):
                    nc.tensor.matmul(
                        ps, lhsT=fT[:, 2 * j:2 * j + 2, qt * P:(qt + 1) * P],
                        rhs=w2_sb[:, 2 * j:2 * j + 2,
                                  half * 512:(half + 1) * 512],
                        start=(j == 0), stop=(j == KH // 2 - 1), perf_mode=DR)
                # undo the w2 fp8 range scale and apply ffn_scale's scalar
                nc.vector.scalar_tensor_tensor(
                    yt[:, half * 512:(half + 1) * 512], ps, io["c_y"],
                    h_sb[:, qt, half * 512:(half + 1) * 512],
                    op0=mybir.AluOpType.mult, op1=mybir.AluOpType.add)
            nc.sync.dma_start(y[qt * P:(qt + 1) * P, :], yt)


def build_nc(c_y: float, c_wo: float):
    nc = bacc.Bacc("TRN2", target_bir_lowering=False, debug=False,
                   num_devices=NCORES)
    io = {
        "xbf": nc.dram_tensor("xbf", [CTX, D], BF16, kind="ExternalInput").ap(),
        "xTf8": nc.dram_tensor("xTf8", [D, CTX], FP8,
                               kind="ExternalInput").ap(),
        "xown": nc.dram_tensor("xown", [OWN, D], F32,
                               kind="ExternalInput").ap(),
        "wqT": nc.dram_tensor("wqT", [D, D], FP8, kind="ExternalInput").ap(),
        "wkvT": nc.dram_tensor("wkvT", [D, 2 * KVH * HD], FP8,
                               kind="ExternalInput").ap(),
        "woT": nc.dram_tensor("woT", [D, D], FP8, kind="ExternalInput").ap(),
        "w1T": nc.dram_tensor("w1T", [D, HID], FP8, kind="ExternalInput").ap(),
        "w3T": nc.dram_tensor("w3T", [D, HID], FP8, kind="ExternalInput").ap(),
        "w2T": nc.dram_tensor("w2T", [HID, D], FP8, kind="ExternalInput").ap(),
        "qw2": nc.dram_tensor("qw2", [P, 1], F32, kind="ExternalInput").ap(),
        "kw2": nc.dram_tensor("kw2", [P, 1], F32, kind="ExternalInput").ap(),
        "tri_g": nc.dram_tensor("tri_g", [P, P], BF16,
                                kind="ExternalInput").ap(),
        "tri_p": nc.dram_tensor("tri_p", [P, P], mybir.dt.uint8,
                                kind="ExternalInput").ap(),
        "vones": nc.dram_tensor("vones", [P, NKT], BF16,
                                kind="ExternalInput").ap(),
        "y": nc.dram_tensor("y", [OWN, D], F32, kind="ExternalOutput").ap(),
        "c_y": c_y,
        "c_wo": c_wo,
    }
    with tile.TileContext(nc) as tc:
        with ExitStack() as ctx:
            _build_tile_kernel(ctx, tc, io)
    nc.compile()
    return nc


_CACHE = {}


def get_nc(c_y: float, c_wo: float):
    if "nc" not in _CACHE:
        _CACHE["nc"] = build_nc(c_y, c_wo)
    return _CACHE["nc"]


def _fp8(a):
    return np.ascontiguousarray(
        np.clip(a, -240.0, 240.0)).astype(NPFP8)


def prep_in_maps(inputs):
    """Fold scales into weights, transpose/cast, and slice per-core inputs."""
    f32 = np.float32
    x = np.asarray(inputs["x"], f32)
    wq = np.asarray(inputs["wq"], f32)
    wk = np.asarray(inputs["wk"], f32)
    wv = np.asarray(inputs["wv"], f32)
    wo = np.asarray(inputs["wo"], f32)
    w1 = np.asarray(inputs["w1"], f32)
    w2 = np.asarray(inputs["w2"], f32)
    w3 = np.asarray(inputs["w3"], f32)
    qw = np.asarray(inputs["q_norm_w"], f32)
    kw = np.asarray(inputs["k_norm_w"], f32)
    anw = np.asarray(inputs["attn_norm_w"], f32)
    fnw = np.asarray(inputs["ffn_norm_w"], f32)
    asc = np.asarray(inputs["attn_scale"], f32)
    fsc = np.asarray(inputs["ffn_scale"], f32)

    HEAD_PERM = [0, 4, 1, 5, 2, 6, 3, 7, 8, 12, 9, 13, 10, 14, 11, 15]
    wq_p = (wq * anw[None, :]).reshape(H, HD, D)[HEAD_PERM].reshape(H * HD, D)
    wqT = _fp8(wq_p.T * S_WQKV)
    wkvT = _fp8(
        np.concatenate([wk * anw[None, :], wv * anw[None, :]], axis=0).T
        * S_WQKV)
    asc_s = float(np.mean(asc))
    c_wo = asc_s / S_WO
    wo_p = ((wo * (asc / np.float32(asc_s))[:, None])
            .T.reshape(H, HD, D)[HEAD_PERM].reshape(H * HD, D))
    woT = _fp8(wo_p * S_WO)
    w1T = _fp8((w1 * fnw[None, :]).T * S_W13)
    w3T = _fp8((w3 * fnw[None, :]).T * S_W13)
    fsc_s = float(np.mean(fsc))
    c_y = fsc_s / S_W2
    w2T = _fp8((w2 * (fsc / np.float32(fsc_s))[:, None]).T * S_W2)
    qwb = np.ascontiguousarray(np.tile(qw, 2)[:, None]).astype(f32)
    kwb = np.ascontiguousarray(np.tile(kw, 2)[:, None]).astype(f32)

    # diagonal-block triangle masks:
    # d0 block (ki==qt): valid iff k > qq (bf16 VALID mask, gpsimd multiply)
    # d4 block (ki==qt+4): valid iff k <= qq (uint8 INVALID mask, DVE zeroing)
    k_i = np.arange(P)[:, None]
    q_i = np.arange(P)[None, :]
    tri_g = np.ascontiguousarray((k_i > q_i).astype(NPBF16))
    tri_p = np.ascontiguousarray((k_i > q_i).astype(np.uint8))

    # per-token validity for V's appended column (0 for halo padding)
    v_int = np.ones((P, NKT), NPBF16)
    v_first = np.zeros((P, NKT), NPBF16)
    v_first[:, NQT:] = 1.0

    shared = dict(wqT=wqT, wkvT=wkvT, woT=woT, w1T=w1T, w3T=w3T, w2T=w2T,
                  qw2=qwb, kw2=kwb, tri_g=tri_g, tri_p=tri_p)
    in_maps = []
    for b in range(B):
        for j in range(T // OWN):
            xc = np.zeros((CTX, D), f32)
            if j == 0:
                xc[OWN:] = x[b, 0:OWN]
                vm = v_first
            else:
                xc[:] = x[b, (j - 1) * OWN:(j + 1) * OWN]
                vm = v_int
            in_maps.append(dict(xbf=xc.astype(NPBF16),
                                xTf8=_fp8(np.ascontiguousarray(xc.T)),
                                xown=np.ascontiguousarray(xc[OWN:]),
                                vones=vm, **shared))
    return in_maps, c_y, c_wo


LAST_RESULTS = None


def _ensure_ntff_hook():
    """Install the axon NTFF profile hook if the image's antenv lacks it."""
    import types
    try:
        from antenv.axon_hooks import get_axon_ntff_profile_hook  # noqa: F401
        return  # real module present
    except ImportError:
        pass
    try:
        import antenv
        boot_dir = "/root/.axon_site/trn_agent_boot"
        if boot_dir not in sys.path:
            sys.path.insert(0, boot_dir)
        import trn_boot
        hook = trn_boot._ntff_profile_via_ctypes("/opt/axon/libaxon_pjrt.so")
        mod = types.ModuleType("antenv.axon_hooks")
        mod._hook = hook
        mod.get_axon_ntff_profile_hook = lambda: mod._hook
        mod.set_axon_ntff_profile_hook = lambda h: setattr(mod, "_hook", h)
        sys.modules["antenv.axon_hooks"] = mod
        antenv.axon_hooks = mod
        import concourse.bass_utils as _bu
        _bu.upload_artifacts = lambda tmpdir: tmpdir
    except Exception as e:  # pragma: no cover
        print(f"ntff hook unavailable ({e}); running without trace")


def kernel(**inputs):
    global LAST_RESULTS
    if os.environ.get("BASS_TRACE"):
        _ensure_ntff_hook()
    in_maps, c_y, c_wo = prep_in_maps(inputs)
    nc = get_nc(c_y, c_wo)
    res = run_bass_kernel_spmd(nc, in_maps, core_ids=list(range(NCORES)))
    LAST_RESULTS = res
    y = np.empty((B, T, D), np.float32)
    for c in range(NCORES):
        b, j = divmod(c, T // OWN)
        y[b, j * OWN:(j + 1) * OWN] = res.results[c]["y"]
    return y


# revision 10
# speedup vs baseline: 1.3161x; 1.0761x over previous
"""Trainium2 Bass kernel for a codec-transformer block (sliding-window GQA + SwiGLU).

Sharding: data-parallel over 8 token chunks (2 batches x 4 chunks of 512
tokens). The 512-token sliding window makes attention local: each core
receives its 512 "own" tokens plus the preceding 512 tokens as a KV halo,
so no collectives are needed.

Host-side prep (layout only, no model FLOPs):
  - attn_norm_w folded into wq/wk/wv columns, ffn_norm_w into w1/w3 columns
  - attn_scale/ffn_scale: their scalar parts are applied on-chip (c_wo, c_y)
    so the fp8 weights keep a healthy range; only the shape (ratio to the
    scalar) is folded into wo/w2 rows
  - wq/wkv/wo/w1/w3/w2 are cast to fp8e4 with power-of-two range scales;
    every scale is absorbed into an op the kernel already needs:
      * wq,wk x32: cancels in qk-rmsnorm (scale-invariant)
      * wv x32: undone by the per-token rstd fold at the V eviction
      * w1,w3 x8: undone by scaling hn by 1/8 (folded into the hn-rmsnorm
        sqrt scale), making psg/psu exact so fT = silu(psg)*psu is exact
      * wo x16 / w2 x16: undone by the c_wo/c_y constants on the h/y paths
  - x is sent twice: bf16 [CTX,D] for the transpose/matmul path and f32
    [OWN,D] for the residual; the x-rmsnorm cancels in qk-norm for Q/K and
    is applied to V via rstd at its eviction, so raw x feeds the PE
    transposes directly (no norm on the critical path).

Attention stage: head pairs (sharing a kv pair-transposed kT2 tile) run
concurrently on PE row groups 0-63/64-127. Scores for one head live in two
[P,1280] PSUM tiles with a ragged ki-permuted layout chosen so every
matmul output stays inside a 2KB bank; exp then runs as ONE activation per
half-head. The sliding-window mask reduces to two constant 128x128
triangles applied only to the two diagonal blocks per query tile
(copy_predicated with zeros); halo-padding tokens are excluded via a
0/1 validity column in V's appended ones-column, which zeroes both their
numerator and softmax-denominator contributions.
"""

import os
import sys

sys.path.insert(0, "/opt/trn_rl_repo")
os.environ.setdefault("MYCRO_LOCAL_CACHE", "1")

from contextlib import ExitStack

import numpy as np
import ml_dtypes

import concourse.bass as bass
import concourse.bacc as bacc
import concourse.tile as tile
from concourse import mybir
from concourse.masks import make_identity
from concourse.bass_utils import run_bass_kernel_spmd

BF16 = mybir.dt.bfloat16
F32 = mybir.dt.float32
FP8 = mybir.dt.float8e4
AF = mybir.ActivationFunctionType
DR = mybir.MatmulPerfMode.DoubleRow
NPBF16 = ml_dtypes.bfloat16
NPFP8 = ml_dtypes.float8_e4m3

P = 128
B, T, D = 2, 2048, 1024
HID = 4096
H, KVH, HD = 16, 4, 64
KD = D // P            # 8 contraction tiles over model dim
KH = HID // P          # 32 contraction tiles over hidden dim
OWN = 512              # tokens owned per core
CTX = 1024             # own + 512-token halo
NQT = OWN // P         # 4
NKT = CTX // P         # 8
NCORES = 8
KC = KVH * HD          # 256
EPS = 1e-5
QKEPS = 1e-6
SM_SCALE = 1.0 / 8.0   # 1/sqrt(HD)

S_WQKV = 32.0          # fp8 range scale on wq/wk/wv
S_W13 = 8.0            # fp8 range scale on w1/w3 (alpha = 1/8 on hn)
S_W2 = 16.0            # fp8 range scale on w2
S_WO = 16.0            # fp8 range scale on wo

# Ragged in-bank PSUM layout for one half-head of scores ([P,1280] f32).
# Widths per ki: 128,256,384,512,512,384,256,128; this permutation keeps
# every matmul output inside a 2KB (512-f32) PSUM bank.
OFF = {0: 896, 1: 1024, 2: 512, 3: 0,
       4: 1280 + 0, 5: 1280 + 512, 6: 1280 + 1024, 7: 1280 + 896}


def _qclip(ki):
    """Valid own-query range for ctx key tile ki under the sliding window."""
    return max(0, P * (ki - 4)), min(OWN, P * (ki + 1))


def _es_col(qt, ki):
    """eS/psum column of query-tile block (qt, ki) in the ragged layout."""
    return OFF[ki] + qt * P - _qclip(ki)[0]


def _build_tile_kernel(ctx: ExitStack, tc: tile.TileContext, io: dict):
    nc = tc.nc
    y = io["y"]

    const = ctx.enter_context(tc.tile_pool(name="const", bufs=1))
    identity = const.tile([P, P], BF16)
    make_identity(nc, identity)
    qw2_sb = const.tile([P, 1], F32)    # q_norm_w tiled over both 64-rows
    nc.sync.dma_start(qw2_sb, io["qw2"])
    kw2_sb = const.tile([P, 1], F32)
    nc.sync.dma_start(kw2_sb, io["kw2"])
    tri_g = const.tile([P, P], BF16)        # d0 VALID mask (k > qq), gpsimd
    nc.sync.dma_start(tri_g, io["tri_g"])
    tri_p = const.tile([P, P], mybir.dt.uint8)  # d4 INVALID mask (k > qq), DVE
    nc.sync.dma_start(tri_p, io["tri_p"])
    zeros_sb = const.tile([P, P], BF16)
    nc.vector.memset(zeros_sb, 0.0)
    vm_sb = const.tile([P, NKT], BF16)      # per-token validity (halo pad=0)
    nc.sync.dma_start(vm_sb, io["vones"])
    epsv_sb = const.tile([P, 1], F32)
    nc.vector.memset(epsv_sb, EPS * S_WQKV * S_WQKV)
    epsh_sb = const.tile([P, 1], F32)
    nc.vector.memset(epsh_sb, EPS * S_W13 * S_W13)
    qkeps_sb = const.tile([P, 1], F32)
    nc.vector.memset(qkeps_sb, QKEPS)

    sstat = ctx.enter_context(tc.tile_pool(name="sstat", bufs=8))

    pers = ctx.enter_context(tc.tile_pool(name="pers", bufs=1))
    h_sb = pers.tile([P, NQT, D], F32)       # residual h = x + r, fp32
    hnT_pool = ctx.enter_context(tc.tile_pool(name="hnT_pool", bufs=1))
    hnT = hnT_pool.tile([P, KD, OWN], FP8)
    wqkv_pool = ctx.enter_context(tc.tile_pool(name="wqkv", bufs=1))
    wkv_sb = wqkv_pool.tile([P, KD, 2 * KC], FP8)
    nc.sync.dma_start(wkv_sb, io["wkvT"].rearrange("(kd p) n -> p kd n", p=P))
    wq_sb = wqkv_pool.tile([P, KD, D], FP8)
    nc.sync.dma_start(wq_sb, io["wqT"].rearrange("(kd p) n -> p kd n", p=P))
    ap_stack = ExitStack()
    attn_pers = ap_stack.enter_context(tc.tile_pool(name="attn_pers", bufs=1))
    # qhat^T: q heads are laid out (via the host-side wq column permutation)
    # so head h lives in feature tile tau=(h%4)+4*(h//8) at partition base
    # pi=((h//4)%2)*64 -- exactly where its kv head lands in kT2's natural
    # pair-transpose layout, so scores operands always share a base partition.
    # Two zero-padded copies of qhat^T: scores run as FULL-K (128-row)
    # matmuls -- the other head-half of the moving operand is zero, so its
    # kT2 rows contribute nothing. Full-row matmuls keep the PE's HAM
    # activity monitor warm (K=8/8) through the attention stage.
    qkT0 = attn_pers.tile([P, KD, OWN], BF16)   # rows 64-127 zero
    qkT1 = attn_pers.tile([P, KD, OWN], BF16)   # rows 0-63 zero
    nc.vector.memset(qkT0[HD:P, :, :], 0.0)
    nc.vector.memset(qkT1[0:HD, :, :], 0.0)
    kT2 = attn_pers.tile([P, 2, CTX], BF16)
    v65 = attn_pers.tile([P, NKT, KVH, HD + 1], BF16)  # v tokens + valid col
    attn_sb = attn_pers.tile([P, NQT, H * HD], BF16)  # attn out, token-major
    for kvh in range(KVH):
        nc.vector.tensor_copy(v65[:, :, kvh, HD:HD + 1], vm_sb[:, :, None])

    xall_stack = ExitStack()
    xall_pool = xall_stack.enter_context(tc.tile_pool(name="xall", bufs=1))
    xbf = xall_pool.tile([P, NKT, D], BF16)
    xown = xall_pool.tile([P, NQT, D], F32)
    xT = xall_pool.tile([P, KD, CTX], FP8)
    for kd in range(KD):
        nc.sync.dma_start(xT[:, kd, :],
                          io["xTf8"][kd * P:(kd + 1) * P, :])
    for i in range(NKT):
        nc.sync.dma_start(xbf[:, i, :], io["xbf"][i * P:(i + 1) * P, :])
    for i in range(NQT):
        nc.sync.dma_start(xown[:, i, :], io["xown"][i * P:(i + 1) * P, :])

    # ---- Stages A+B: x transpose + QKV (fp8 DoubleRow), per ctx tile ----
    stage_a = ExitStack()
    with stage_a:
        tp_ps = stage_a.enter_context(
            tc.tile_pool(name="tp_ps", bufs=3, space="PSUM"))
        pb_ps = stage_a.enter_context(
            tc.tile_pool(name="pb_ps", bufs=5, space="PSUM"))
        pb = stage_a.enter_context(tc.tile_pool(name="pb", bufs=3))

        def stats_tile(i):
            """sum(x^2) -> rstd_v = 1/(32*sqrt(mean+eps)) (ACT/DVE only)."""
            sq = pb.tile([P, D], F32, tag="sq")
            ssq = sstat.tile([P, 1], F32, tag="ssq")
            nc.scalar.activation(sq, xbf[:, i, :], AF.Square, accum_out=ssq)
            stdv = sstat.tile([P, 1], F32, tag="std")
            nc.scalar.activation(stdv, ssq, AF.Sqrt, bias=epsv_sb,
                                 scale=S_WQKV * S_WQKV / D)
            rstd_v = sstat.tile([P, 1], F32, tag="rstdv")
            nc.vector.reciprocal(rstd_v, stdv)
            return rstd_v

        def emit_k_tp(kt, khat):
            # eviction applies k_norm_w (per feature = per partition here)
            pt = tp_ps.tile([P, 2, P], BF16, tag="tp")
            for kf in range(2):
                nc.tensor.transpose(pt[:, kf, :],
                                    khat[:, kf * P:(kf + 1) * P], identity)
            nc.vector.tensor_scalar_mul(
                kT2[:, :, kt * P:(kt + 1) * P], pt, kw2_sb)

        def emit_q_tp(qt, qhats):
            for half in range(2):
                for j in range(0, 4, 2):
                    pt = tp_ps.tile([P, 2, P], BF16, tag="tp")
                    nc.tensor.transpose(
                        pt[:, 0, :], qhats[half][:, j * P:(j + 1) * P],
                        identity)
                    nc.tensor.transpose(
                        pt[:, 1, :], qhats[half][:, (j + 1) * P:(j + 2) * P],
                        identity)
                    nc.vector.tensor_scalar_mul(
                        qkT0[0:HD, half * 4 + j:half * 4 + j + 2,
                             qt * P:(qt + 1) * P], pt[0:HD], qw2_sb[0:HD])
                    nc.vector.tensor_scalar_mul(
                        qkT1[HD:P, half * 4 + j:half * 4 + j + 2,
                             qt * P:(qt + 1) * P], pt[HD:P], qw2_sb[HD:P])

        # kq-hat transposes run TWO tiles behind their matmuls so the
        # qk-norm ACT/DVE chains never stall the PE stream.
        rstds = {0: stats_tile(0), 1: stats_tile(1)}
        pend_k = {}
        pend_q = {}
        for i in range(NKT):
            rstd_v = rstds.pop(i)
            # K / V projection for ctx tile i (fp8 DoubleRow over kd pairs)
            ps = pb_ps.tile([P, 512], F32, tag="ps")
            for j in range(KD // 2):
                nc.tensor.matmul(
                    ps, lhsT=xT[:, 2 * j:2 * j + 2, i * P:(i + 1) * P],
                    rhs=wkv_sb[:, 2 * j:2 * j + 2, :],
                    start=(j == 0), stop=(j == KD // 2 - 1), perf_mode=DR)
            kv_ps = ps

            # Q projection for own tile qt = i - 4
            q_pss = None
            if i >= NQT:
                qt = i - NQT
                col = OWN + qt * P
                q_pss = []
                for half in range(2):
                    ps = pb_ps.tile([P, 512], F32, tag="ps")
                    q_pss.append(ps)
                    for j in range(KD // 2):
                        nc.tensor.matmul(
                            ps, lhsT=xT[:, 2 * j:2 * j + 2, col:col + P],
                            rhs=wq_sb[:, 2 * j:2 * j + 2,
                                      half * 512:(half + 1) * 512],
                            start=(j == 0), stop=(j == KD // 2 - 1),
                            perf_mode=DR)

            # two-behind transposes keep the PE stream dense
            if i - 2 in pend_k:
                emit_k_tp(i - 2, pend_k.pop(i - 2))
            if i - 2 - NQT in pend_q:
                emit_q_tp(i - 2 - NQT, pend_q.pop(i - 2 - NQT))

            # k-chain + v eviction (ACT/DVE)
            ps = kv_ps
            sqk = pb.tile([P, KC], F32, tag="sqk")
            nc.scalar.activation(sqk, ps[:, 0:KC], AF.Square)
            msk = pb.tile([P, KVH], F32, tag="msk")
            nc.vector.reduce_sum(
                msk, sqk.rearrange("p (h e) -> p h e", e=HD),
                axis=mybir.AxisListType.X)
            sck = sstat.tile([P, KVH], F32, tag="sck")
            nc.scalar.activation(sck, msk, AF.Sqrt, bias=qkeps_sb, scale=1.0 / HD)
            rck = sstat.tile([P, KVH], F32, tag="rck")
            nc.vector.reciprocal(rck, sck)
            khat = pb.tile([P, KC], BF16, tag="khat")
            nc.vector.tensor_mul(
                khat.rearrange("p (h e) -> p h e", e=HD),
                ps[:, 0:KC].rearrange("p (h e) -> p h e", e=HD),
                rck[:, :, None].broadcast_to([P, KVH, HD]))
            pend_k[i] = khat
            nc.vector.tensor_scalar_mul(
                v65[:, i, :, 0:HD],
                ps[:, KC:2 * KC].rearrange("p (h e) -> p h e", e=HD),
                rstd_v)

            # q-chain
            if q_pss is not None:
                qt = i - NQT
                msq = pb.tile([P, H], F32, tag="msq")
                for half in range(2):
                    sqq = pb.tile([P, 512], F32, tag="sqq")
                    nc.scalar.activation(sqq, q_pss[half], AF.Square)
                    nc.vector.reduce_sum(
                        msq[:, half * 8:(half + 1) * 8],
                        sqq.rearrange("p (h e) -> p h e", e=HD),
                        axis=mybir.AxisListType.X)
                sc = sstat.tile([P, H], F32, tag="sc")
                nc.scalar.activation(sc, msq, AF.Sqrt, bias=qkeps_sb,
                                     scale=1.0 / HD)
                rc = sstat.tile([P, H], F32, tag="rc")
                nc.vector.reciprocal(rc, sc)
                qhats = []
                for half in range(2):
                    ps = q_pss[half]
                    qhat = pb.tile([P, 512], BF16, tag="qhat")
                    nc.vector.tensor_mul(
                        qhat.rearrange("p (h e) -> p h e", e=HD),
                        ps.rearrange("p (h e) -> p h e", e=HD),
                        rc[:, half * 8:(half + 1) * 8, None]
                        .broadcast_to([P, 8, HD]))
                    qhats.append(qhat)
                pend_q[qt] = qhats

            if i + 2 < NKT:
                rstds[i + 2] = stats_tile(i + 2)

        for i in (NKT - 2, NKT - 1):
            emit_k_tp(i, pend_k.pop(i))
        for qt in (NQT - 2, NQT - 1):
            emit_q_tp(qt, pend_q.pop(qt))

    # ---- Stage C: attention. Head pairs run on PE row groups 0/64. ----
    stage_c = ExitStack()
    with stage_c:
        es_pool = stage_c.enter_context(tc.tile_pool(name="es_pool", bufs=2))
        psc = stage_c.enter_context(
            tc.tile_pool(name="psc", bufs=1, space="PSUM"))
        ps_o = stage_c.enter_context(
            tc.tile_pool(name="ps_o", bufs=2, space="PSUM"))

        def emit_pv(h, eS, eSd):
            kvh = h // 4
            tau = (h % 4) + 4 * (h // 8)
            pi = ((h // 4) % 2)
            slot = 2 * tau + pi
            for qt in range(NQT):
                po = ps_o.tile([P, HD + 1], F32, tag="po")
                for j in range(5):
                    if j == 0:
                        lhs = eSd[:, qt, :]
                    else:
                        c = _es_col(qt, qt + j)
                        lhs = eS[:, c:c + P]
                    nc.tensor.matmul(
                        po, lhsT=lhs,
                        rhs=v65[:, qt + j, kvh, :],
                        start=(j == 0), stop=(j == 4))
                rec = sstat.tile([P, 1], F32, tag="rec")
                nc.vector.reciprocal(rec, po[:, HD:HD + 1])
                nc.vector.tensor_scalar_mul(
                    attn_sb[:, qt, slot * HD:(slot + 1) * HD], po[:, 0:HD],
                    rec)

        PAIRS = [(0, 4), (1, 5), (2, 6), (3, 7),
                 (8, 12), (9, 13), (10, 14), (11, 15)]
        pending = []
        for hA, hB in PAIRS:
            g = hA // 8
            tau = (hA % 4) + 4 * (hA // 8)
            eS_A = es_pool.tile([P, 2 * 1280], BF16, tag="esA")
            eS_B = es_pool.tile([P, 2 * 1280], BF16, tag="esB")
            eSd_A = es_pool.tile([P, NQT, P], BF16, tag="esdA")
            eSd_B = es_pool.tile([P, NQT, P], BF16, tag="esdB")
            for half in range(2):
                psA = psc.tile([P, 1280], F32, tag="psA")
                psB = psc.tile([P, 1280], F32, tag="psB")
                for ki in range(half * 4, half * 4 + 4):
                    qlo, qhi = _qclip(ki)
                    w = qhi - qlo
                    o = OFF[ki] - half * 1280
                    nc.tensor.matmul(
                        psA[:, o:o + w],
                        lhsT=kT2[:, g, ki * P:(ki + 1) * P],
                        rhs=qkT0[:, tau, qlo:qhi],
                        start=True, stop=True)
                    nc.tensor.matmul(
                        psB[:, o:o + w],
                        lhsT=kT2[:, g, ki * P:(ki + 1) * P],
                        rhs=qkT1[:, tau, qlo:qhi],
                        start=True, stop=True)
                for eS, psX in ((eS_A, psA), (eS_B, psB)):
                    nc.scalar.activation(
                        eS[:, half * 1280:(half + 1) * 1280], psX, AF.Exp,
                        scale=SM_SCALE)
                # diagonal-block masks: d0 via gpsimd into eSd (PV reads
                # eSd), d4 zeroed inside eS via a predicated write (DVE)
                for qt in range(NQT):
                    if half == 0:
                        c = _es_col(qt, qt)
                        for eS, eSd in ((eS_A, eSd_A), (eS_B, eSd_B)):
                            nc.gpsimd.tensor_mul(
                                eSd[:, qt, :], eS[:, c:c + P], tri_g)
                    else:
                        c = _es_col(qt, qt + 4)
                        for eS in (eS_A, eS_B):
                            nc.vector.copy_predicated(
                                eS[:, c:c + P], tri_p, zeros_sb)
                # previous pair's PV fills the PE while exp drains psA/psB
                if pending:
                    emit_pv(pending.pop(0), pending.pop(0), pending.pop(0))
            pending = [hA, eS_A, eSd_A, hB, eS_B, eSd_B]
        emit_pv(pending.pop(0), pending.pop(0), pending.pop(0))
        emit_pv(pending.pop(0), pending.pop(0), pending.pop(0))

    # ---- Stages D+E: attnT transpose + wo (fp8 DR) + residual + ffn norm ----
    attnT_stack = ExitStack()
    attnT_pool = attnT_stack.enter_context(tc.tile_pool(name="attnT_pool",
                                                        bufs=1))
    attnT = attnT_pool.tile([P, KD, OWN], FP8)

    stage_de = ExitStack()
    with stage_de:
        wo_pool = stage_de.enter_context(tc.tile_pool(name="wo_pool", bufs=1))
        wo_sb = wo_pool.tile([P, KD, D], FP8)
        nc.sync.dma_start(wo_sb, io["woT"].rearrange("(kd p) n -> p kd n", p=P))
        ps_r = stage_de.enter_context(
            tc.tile_pool(name="ps_r", bufs=2, space="PSUM"))
        tp_d = stage_de.enter_context(
            tc.tile_pool(name="tp_d", bufs=3, space="PSUM"))
        tp_ps3 = stage_de.enter_context(
            tc.tile_pool(name="tp_ps3", bufs=2, space="PSUM"))
        pe = stage_de.enter_context(tc.tile_pool(name="pe", bufs=2))

        def emit_attnT(qt):
            for kd in range(KD):
                pt = tp_d.tile([P, P], BF16, tag="tpd")
                nc.tensor.transpose(pt, attn_sb[:, qt, kd * P:(kd + 1) * P],
                                    identity)
                nc.vector.tensor_copy(attnT[:, kd, qt * P:(qt + 1) * P], pt)

        emit_attnT(0)
        emit_attnT(1)
        pend_hn = None
        for qt in range(NQT):
            xr = xown[:, qt, :]
            for half in range(2):
                ps = ps_r.tile([P, 512], F32, tag="psr")
                for j in range(KD // 2):
                    nc.tensor.matmul(
                        ps, lhsT=attnT[:, 2 * j:2 * j + 2, qt * P:(qt + 1) * P],
                        rhs=wo_sb[:, 2 * j:2 * j + 2,
                                  half * 512:(half + 1) * 512],
                        start=(j == 0), stop=(j == KD // 2 - 1), perf_mode=DR)
                nc.vector.scalar_tensor_tensor(
                    h_sb[:, qt, half * 512:(half + 1) * 512], ps, io["c_wo"],
                    xr[:, half * 512:(half + 1) * 512],
                    op0=mybir.AluOpType.mult, op1=mybir.AluOpType.add)
            if qt + 2 < NQT:
                emit_attnT(qt + 2)
            if pend_hn is not None:
                pqt, phn = pend_hn
                for kd in range(KD):
                    pt = tp_ps3.tile([P, P], BF16, tag="tp3")
                    nc.tensor.transpose(pt, phn[:, kd * P:(kd + 1) * P],
                                        identity)
                    nc.vector.tensor_copy(hnT[:, kd, pqt * P:(pqt + 1) * P], pt)
            # ffn rmsnorm; hn is scaled by 1/8 to undo the w1/w3 fp8 scale
            sqh = pe.tile([P, D], F32, tag="sqh")
            ssqh = sstat.tile([P, 1], F32, tag="ssq")
            nc.scalar.activation(sqh, h_sb[:, qt, :], AF.Square, accum_out=ssqh)
            stdh = sstat.tile([P, 1], F32, tag="std")
            nc.scalar.activation(stdh, ssqh, AF.Sqrt, bias=epsh_sb,
                                 scale=S_W13 * S_W13 / D)
            rstdh = sstat.tile([P, 1], F32, tag="rstd")
            nc.vector.reciprocal(rstdh, stdh)
            hn = pe.tile([P, D], BF16, tag="hn")
            nc.vector.tensor_scalar_mul(hn, h_sb[:, qt, :], rstdh)
            pend_hn = (qt, hn)
        pqt, phn = pend_hn
        for kd in range(KD):
            pt = tp_ps3.tile([P, P], BF16, tag="tp3")
            nc.tensor.transpose(pt, phn[:, kd * P:(kd + 1) * P], identity)
            nc.vector.tensor_copy(hnT[:, kd, pqt * P:(pqt + 1) * P], pt)

    attnT_stack.close()
    xall_stack.close()
    ap_stack.close()

    # ---- Stage F: SwiGLU FFN (fp8 DoubleRow) ----
    stage_f = ExitStack()
    with stage_f:
        fT_pool = stage_f.enter_context(tc.tile_pool(name="fT_pool", bufs=1))
        fT = fT_pool.tile([P, KH, OWN], FP8)   # silu(g) * u, feature-major
        w13 = stage_f.enter_context(tc.tile_pool(name="w13", bufs=4))
        ps_f = stage_f.enter_context(
            tc.tile_pool(name="ps_f", bufs=2, space="PSUM"))
        pf = stage_f.enter_context(tc.tile_pool(name="pf", bufs=2))

        for mi in range(KH):
            w1t = w13.tile([P, KD, P], FP8, tag="w1t")
            nc.sync.dma_start(
                w1t, io["w1T"][:, mi * P:(mi + 1) * P]
                .rearrange("(kd p) m -> p kd m", p=P))
            w3t = w13.tile([P, KD, P], FP8, tag="w3t")
            nc.sync.dma_start(
                w3t, io["w3T"][:, mi * P:(mi + 1) * P]
                .rearrange("(kd p) m -> p kd m", p=P))
            psg = ps_f.tile([P, 512], F32, tag="pg")
            for j in range(KD // 2):
                nc.tensor.matmul(psg, lhsT=w1t[:, 2 * j:2 * j + 2, :],
                                 rhs=hnT[:, 2 * j:2 * j + 2, :],
                                 start=(j == 0), stop=(j == KD // 2 - 1),
                                 perf_mode=DR)
            psu = ps_f.tile([P, 512], F32, tag="pu")
            for j in range(KD // 2):
                nc.tensor.matmul(psu, lhsT=w3t[:, 2 * j:2 * j + 2, :],
                                 rhs=hnT[:, 2 * j:2 * j + 2, :],
                                 start=(j == 0), stop=(j == KD // 2 - 1),
                                 perf_mode=DR)
            # psg/psu are exact g/u (scales cancelled): silu via sigmoid
            sg = pf.tile([P, 512], F32, tag="sg")
            nc.scalar.activation(sg, psg, AF.Sigmoid)
            gm = pf.tile([P, 512], F32, tag="gm")
            nc.vector.tensor_mul(gm, sg, psg)
            nc.vector.tensor_mul(fT[:, mi, :], gm, psu)

        w2_pool = stage_f.enter_context(tc.tile_pool(name="w2_pool", bufs=1))
        w2_sb = w2_pool.tile([P, KH, D], FP8)
        for c in range(8):
            nc.sync.dma_start(
                w2_sb[:, c * 4:(c + 1) * 4, :],
                io["w2T"][c * 4 * P:(c + 1) * 4 * P, :]
                .rearrange("(kh p) n -> p kh n", p=P))
        ps_y = stage_f.enter_context(
            tc.tile_pool(name="ps_y", bufs=2, space="PSUM"))
        py = stage_f.enter_context(tc.tile_pool(name="py", bufs=2))

        for qt in range(NQT):
            yt = py.tile([P, D], F32, tag="yt")
            for half in range(2):
                ps = ps_y.tile([P, 512], F32, tag="psy")
                for j in range(KH // 2):
                    nc.tensor.matmul(
                        ps, lhsT=fT[:, 2 * j:2 * j + 2, qt * P:(qt + 1) * P],
                        rhs=w2_sb[:, 2 * j:2 * j + 2,
                                  half * 512:(half + 1) * 512],
                        start=(j == 0), stop=(j == KH // 2 - 1), perf_mode=DR)
                # undo the w2 fp8 range scale and apply ffn_scale's scalar
                nc.vector.scalar_tensor_tensor(
                    yt[:, half * 512:(half + 1) * 512], ps, io["c_y"],
                    h_sb[:, qt, half * 512:(half + 1) * 512],
                    op0=mybir.AluOpType.mult, op1=mybir.AluOpType.add)
            nc.sync.dma_start(y[qt * P:(qt + 1) * P, :], yt)


def build_nc(c_y: float, c_wo: float):
    nc = bacc.Bacc("TRN2", target_bir_lowering=False, debug=False,
                   num_devices=NCORES)
    io = {
        "xbf": nc.dram_tensor("xbf", [CTX, D], BF16, kind="ExternalInput").ap(),
        "xTf8": nc.dram_tensor("xTf8", [D, CTX], FP8,
                               kind="ExternalInput").ap(),
        "xown": nc.dram_tensor("xown", [OWN, D], F32,
                               kind="ExternalInput").ap(),
        "wqT": nc.dram_tensor("wqT", [D, D], FP8, kind="ExternalInput").ap(),
        "wkvT": nc.dram_tensor("wkvT", [D, 2 * KVH * HD], FP8,
                               kind="ExternalInput").ap(),
        "woT": nc.dram_tensor("woT", [D, D], FP8, kind="ExternalInput").ap(),
        "w1T": nc.dram_tensor("w1T", [D, HID], FP8, kind="ExternalInput").ap(),
        "w3T": nc.dram_tensor("w3T", [D, HID], FP8, kind="ExternalInput").ap(),
        "w2T": nc.dram_tensor("w2T", [HID, D], FP8, kind="ExternalInput").ap(),
        "qw2": nc.dram_tensor("qw2", [P, 1], F32, kind="ExternalInput").ap(),
        "kw2": nc.dram_tensor("kw2", [P, 1], F32, kind="ExternalInput").ap(),
        "tri_g": nc.dram_tensor("tri_g", [P, P], BF16,
                                kind="ExternalInput").ap(),
        "tri_p": nc.dram_tensor("tri_p", [P, P], mybir.dt.uint8,
                                kind="ExternalInput").ap(),
        "vones": nc.dram_tensor("vones", [P, NKT], BF16,
                                kind="ExternalInput").ap(),
        "y": nc.dram_tensor("y", [OWN, D], F32, kind="ExternalOutput").ap(),
        "c_y": c_y,
        "c_wo": c_wo,
    }
    with tile.TileContext(nc) as tc:
        with ExitStack() as ctx:
            _build_tile_kernel(ctx, tc, io)
    nc.compile()
    return nc


_CACHE = {}


def get_nc(c_y: float, c_wo: float):
    if "nc" not in _CACHE:
        _CACHE["nc"] = build_nc(c_y, c_wo)
    return _CACHE["nc"]


def _fp8(a):
    return np.ascontiguousarray(
        np.clip(a, -240.0, 240.0)).astype(NPFP8)


def prep_in_maps(inputs):
    """Fold scales into weights, transpose/cast, and slice per-core inputs."""
    f32 = np.float32
    x = np.asarray(inputs["x"], f32)
    wq = np.asarray(inputs["wq"], f32)
    wk = np.asarray(inputs["wk"], f32)
    wv = np.asarray(inputs["wv"], f32)
    wo = np.asarray(inputs["wo"], f32)
    w1 = np.asarray(inputs["w1"], f32)
    w2 = np.asarray(inputs["w2"], f32)
    w3 = np.asarray(inputs["w3"], f32)
    qw = np.asarray(inputs["q_norm_w"], f32)
    kw = np.asarray(inputs["k_norm_w"], f32)
    anw = np.asarray(inputs["attn_norm_w"], f32)
    fnw = np.asarray(inputs["ffn_norm_w"], f32)
    asc = np.asarray(inputs["attn_scale"], f32)
    fsc = np.asarray(inputs["ffn_scale"], f32)

    HEAD_PERM = [0, 4, 1, 5, 2, 6, 3, 7, 8, 12, 9, 13, 10, 14, 11, 15]
    wq_p = (wq * anw[None, :]).reshape(H, HD, D)[HEAD_PERM].reshape(H * HD, D)
    wqT = _fp8(wq_p.T * S_WQKV)
    wkvT = _fp8(
        np.concatenate([wk * anw[None, :], wv * anw[None, :]], axis=0).T
        * S_WQKV)
    asc_s = float(np.mean(asc))
    c_wo = asc_s / S_WO
    wo_p = ((wo * (asc / np.float32(asc_s))[:, None])
            .T.reshape(H, HD, D)[HEAD_PERM].reshape(H * HD, D))
    woT = _fp8(wo_p * S_WO)
    w1T = _fp8((w1 * fnw[None, :]).T * S_W13)
    w3T = _fp8((w3 * fnw[None, :]).T * S_W13)
    fsc_s = float(np.mean(fsc))
    c_y = fsc_s / S_W2
    w2T = _fp8((w2 * (fsc / np.float32(fsc_s))[:, None]).T * S_W2)
    qwb = np.ascontiguousarray(np.tile(qw, 2)[:, None]).astype(f32)
    kwb = np.ascontiguousarray(np.tile(kw, 2)[:, None]).astype(f32)

    # diagonal-block triangle masks:
    # d0 block (ki==qt): valid iff k > qq (bf16 VALID mask, gpsimd multiply)
    # d4 block (ki==qt+4): valid iff k <= qq (uint8 INVALID mask, DVE zeroing)
    k_i = np.arange(P)[:, None]
    q_i = np.arange(P)[None, :]
    tri_g = np.ascontiguousarray((k_i > q_i).astype(NPBF16))
    tri_p = np.ascontiguousarray((k_i > q_i).astype(np.uint8))

    # per-token validity for V's appended column (0 for halo padding)
    v_int = np.ones((P, NKT), NPBF16)
    v_first = np.zeros((P, NKT), NPBF16)
    v_first[:, NQT:] = 1.0

    shared = dict(wqT=wqT, wkvT=wkvT, woT=woT, w1T=w1T, w3T=w3T, w2T=w2T,
                  qw2=qwb, kw2=kwb, tri_g=tri_g, tri_p=tri_p)
    in_maps = []
    for b in range(B):
        for j in range(T // OWN):
            xc = np.zeros((CTX, D), f32)
            if j == 0:
                xc[OWN:] = x[b, 0:OWN]
                vm = v_first
            else:
                xc[:] = x[b, (j - 1) * OWN:(j + 1) * OWN]
                vm = v_int
            in_maps.append(dict(xbf=xc.astype(NPBF16),
                                xTf8=_fp8(np.ascontiguousarray(xc.T)),
                                xown=np.ascontiguousarray(xc[OWN:]),
                                vones=vm, **shared))
    return in_maps, c_y, c_wo


LAST_RESULTS = None


def _ensure_ntff_hook():
    """Install the axon NTFF profile hook if the image's antenv lacks it."""
    import types
    try:
        from antenv.axon_hooks import get_axon_ntff_profile_hook  # noqa: F401
        return  # real module present
    except ImportError:
        pass
    try:
        import antenv
        boot_dir = "/root/.axon_site/trn_agent_boot"
        if boot_dir not in sys.path:
            sys.path.insert(0, boot_dir)
        import trn_boot
        hook = trn_boot._ntff_profile_via_ctypes("/opt/axon/libaxon_pjrt.so")
        mod = types.ModuleType("antenv.axon_hooks")
        mod._hook = hook
        mod.get_axon_ntff_profile_hook = lambda: mod._hook
        mod.set_axon_ntff_profile_hook = lambda h: setattr(mod, "_hook", h)
        sys.modules["antenv.axon_hooks"] = mod
        antenv.axon_hooks = mod
        import concourse.bass_utils as _bu
        _bu.upload_artifacts = lambda tmpdir: tmpdir
    except Exception as e:  # pragma: no cover
        print(f"ntff hook unavailable ({e}); running without trace")


def kernel(**inputs):
    global LAST_RESULTS
    if os.environ.get("BASS_TRACE"):
        _ensure_ntff_hook()
    in_maps, c_y, c_wo = prep_in_maps(inputs)
    nc = get_nc(c_y, c_wo)
    res = run_bass_kernel_spmd(nc, in_maps, core_ids=list(range(NCORES)))
    LAST_RESULTS = res
    y = np.empty((B, T, D), np.float32)
    for c in range(NCORES):
        b, j = divmod(c, T // OWN)
        y[b, j * OWN:(j + 1) * OWN] = res.results[c]["y"]
    return y


# revision 12
# speedup vs baseline: 1.3616x; 1.0346x over previous
"""Trainium2 Bass kernel for a codec-transformer block (sliding-window GQA + SwiGLU).

Sharding: data-parallel over 8 token chunks (2 batches x 4 chunks of 512
tokens). The 512-token sliding window makes attention local: each core
receives its 512 "own" tokens plus the preceding 512 tokens as a KV halo,
so no collectives are needed.

Host-side prep (layout only, no model FLOPs):
  - attn_norm_w folded into wq/wk/wv columns, ffn_norm_w into w1/w3 columns
  - attn_scale/ffn_scale: their scalar parts are applied on-chip (c_wo, c_y)
    so the fp8 weights keep a healthy range; only the shape (ratio to the
    scalar) is folded into wo/w2 rows
  - wq/wkv/wo/w1/w3/w2 are cast to fp8e4 with power-of-two range scales;
    every scale is absorbed into an op the kernel already needs:
      * wq,wk x32: cancels in qk-rmsnorm (scale-invariant)
      * wv x32: undone by the per-token rstd fold at the V eviction
      * w1,w3 x8: undone by scaling hn by 1/8 (folded into the hn-rmsnorm
        sqrt scale), making psg/psu exact so fT = silu(psg)*psu is exact
      * wo x16 / w2 x16: undone by the c_wo/c_y constants on the h/y paths
  - x is sent twice: bf16 [CTX,D] for the transpose/matmul path and f32
    [OWN,D] for the residual; the x-rmsnorm cancels in qk-norm for Q/K and
    is applied to V via rstd at its eviction, so raw x feeds the PE
    transposes directly (no norm on the critical path).

Attention stage: head pairs (sharing a kv pair-transposed kT2 tile) run
concurrently on PE row groups 0-63/64-127. Scores for one head live in two
[P,1280] PSUM tiles with a ragged ki-permuted layout chosen so every
matmul output stays inside a 2KB bank; exp then runs as ONE activation per
half-head. The sliding-window mask reduces to two constant 128x128
triangles applied only to the two diagonal blocks per query tile
(copy_predicated with zeros); halo-padding tokens are excluded via a
0/1 validity column in V's appended ones-column, which zeroes both their
numerator and softmax-denominator contributions.
"""

import os
import sys

sys.path.insert(0, "/opt/trn_rl_repo")
os.environ.setdefault("MYCRO_LOCAL_CACHE", "1")

from contextlib import ExitStack

import numpy as np
import ml_dtypes

import concourse.bass as bass
import concourse.bacc as bacc
import concourse.tile as tile
from concourse import mybir
from concourse.masks import make_identity
from concourse.bass_utils import run_bass_kernel_spmd

BF16 = mybir.dt.bfloat16
F32 = mybir.dt.float32
FP8 = mybir.dt.float8e4
AF = mybir.ActivationFunctionType
DR = mybir.MatmulPerfMode.DoubleRow
NPBF16 = ml_dtypes.bfloat16
NPFP8 = ml_dtypes.float8_e4m3

P = 128
B, T, D = 2, 2048, 1024
HID = 4096
H, KVH, HD = 16, 4, 64
KD = D // P            # 8 contraction tiles over model dim
KH = HID // P          # 32 contraction tiles over hidden dim
OWN = 512              # tokens owned per core
CTX = 1024             # own + 512-token halo
NQT = OWN // P         # 4
NKT = CTX // P         # 8
NCORES = 8
KC = KVH * HD          # 256
EPS = 1e-5
QKEPS = 1e-6
SM_SCALE = 1.0 / 8.0   # 1/sqrt(HD)

S_WQKV = 32.0          # fp8 range scale on wq/wk/wv
S_W13 = 8.0            # fp8 range scale on w1/w3 (alpha = 1/8 on hn)
S_W2 = 16.0            # fp8 range scale on w2
S_WO = 16.0            # fp8 range scale on wo

# Ragged in-bank PSUM layout for one half-head of scores ([P,1280] f32).
# Widths per ki: 128,256,384,512,512,384,256,128; this permutation keeps
# every matmul output inside a 2KB (512-f32) PSUM bank.
OFF = {0: 896, 1: 1024, 2: 512, 3: 0,
       4: 1280 + 0, 5: 1280 + 512, 6: 1280 + 1024, 7: 1280 + 896}


def _qclip(ki):
    """Valid own-query range for ctx key tile ki under the sliding window."""
    return max(0, P * (ki - 4)), min(OWN, P * (ki + 1))


def _es_col(qt, ki):
    """eS/psum column of query-tile block (qt, ki) in the ragged layout."""
    return OFF[ki] + qt * P - _qclip(ki)[0]


def _build_tile_kernel(ctx: ExitStack, tc: tile.TileContext, io: dict):
    nc = tc.nc
    y = io["y"]

    const = ctx.enter_context(tc.tile_pool(name="const", bufs=1))
    identity = const.tile([P, P], BF16)
    make_identity(nc, identity)
    qw2_sb = const.tile([P, 1], F32)    # q_norm_w tiled over both 64-rows
    nc.sync.dma_start(qw2_sb, io["qw2"])
    kw2_sb = const.tile([P, 1], F32)
    nc.sync.dma_start(kw2_sb, io["kw2"])
    tri_g = const.tile([P, P], BF16)        # d0 VALID mask (k > qq), gpsimd
    nc.sync.dma_start(tri_g, io["tri_g"])
    tri_p = const.tile([P, P], mybir.dt.uint8)  # d4 INVALID mask (k > qq), DVE
    nc.sync.dma_start(tri_p, io["tri_p"])
    zeros_sb = const.tile([P, P], BF16)
    nc.vector.memset(zeros_sb, 0.0)
    vm_sb = const.tile([P, NKT], BF16)      # per-token validity (halo pad=0)
    nc.sync.dma_start(vm_sb, io["vones"])
    epsv_sb = const.tile([P, 1], F32)
    nc.vector.memset(epsv_sb, EPS * S_WQKV * S_WQKV)
    epsh_sb = const.tile([P, 1], F32)
    nc.vector.memset(epsh_sb, EPS * S_W13 * S_W13)
    qkeps_sb = const.tile([P, 1], F32)
    nc.vector.memset(qkeps_sb, QKEPS)

    sstat = ctx.enter_context(tc.tile_pool(name="sstat", bufs=8))

    pers = ctx.enter_context(tc.tile_pool(name="pers", bufs=1))
    h_sb = pers.tile([P, NQT, D], F32)       # residual h = x + r, fp32
    hnT_pool = ctx.enter_context(tc.tile_pool(name="hnT_pool", bufs=1))
    hnT = hnT_pool.tile([P, KD, OWN], FP8)
    wqkv_pool = ctx.enter_context(tc.tile_pool(name="wqkv", bufs=1))
    wkv_sb = wqkv_pool.tile([P, KD, 2 * KC], FP8)
    for c in range(4):
        nc.sync.dma_start(
            wkv_sb[:, 2 * c:2 * c + 2, :],
            io["wkvT"][2 * c * P:2 * (c + 1) * P, :]
            .rearrange("(kd p) n -> p kd n", p=P))
    wq_sb = wqkv_pool.tile([P, KD, D], FP8)
    for c in range(4):
        nc.sync.dma_start(
            wq_sb[:, 2 * c:2 * c + 2, :],
            io["wqT"][2 * c * P:2 * (c + 1) * P, :]
            .rearrange("(kd p) n -> p kd n", p=P))
    ap_stack = ExitStack()
    attn_pers = ap_stack.enter_context(tc.tile_pool(name="attn_pers", bufs=1))
    # qhat^T: q heads are laid out (via the host-side wq column permutation)
    # so head h lives in feature tile tau=(h%4)+4*(h//8) at partition base
    # pi=((h//4)%2)*64 -- exactly where its kv head lands in kT2's natural
    # pair-transpose layout, so scores operands always share a base partition.
    # Two zero-padded copies of qhat^T: scores run as FULL-K (128-row)
    # matmuls -- the other head-half of the moving operand is zero, so its
    # kT2 rows contribute nothing. Full-row matmuls keep the PE's HAM
    # activity monitor warm (K=8/8) through the attention stage.
    qkT0 = attn_pers.tile([P, KD, OWN], BF16)   # rows 64-127 zero
    qkT1 = attn_pers.tile([P, KD, OWN], BF16)   # rows 0-63 zero
    nc.vector.memset(qkT0[HD:P, :, :], 0.0)
    nc.vector.memset(qkT1[0:HD, :, :], 0.0)
    kT2 = attn_pers.tile([P, 2, CTX], BF16)
    v65 = attn_pers.tile([P, NKT, KVH, HD + 1], BF16)  # v tokens + valid col
    attn_sb = attn_pers.tile([P, NQT, H * HD], BF16)  # attn out, token-major
    for kvh in range(KVH):
        nc.vector.tensor_copy(v65[:, :, kvh, HD:HD + 1], vm_sb[:, :, None])

    xall_stack = ExitStack()
    xall_pool = xall_stack.enter_context(tc.tile_pool(name="xall", bufs=1))
    xbf = xall_pool.tile([P, NKT, D], BF16)
    xown = xall_pool.tile([P, NQT, D], F32)
    xT = xall_pool.tile([P, KD, CTX], FP8)
    for kd in range(KD):
        nc.sync.dma_start(xT[:, kd, :],
                          io["xTf8"][kd * P:(kd + 1) * P, :])
    for i in range(NKT):
        nc.sync.dma_start(xbf[:, i, :], io["xbf"][i * P:(i + 1) * P, :])
    for i in range(NQT):
        nc.sync.dma_start(xown[:, i, :], io["xown"][i * P:(i + 1) * P, :])

    # ---- Stages A+B: x transpose + QKV (fp8 DoubleRow), per ctx tile ----
    stage_a = ExitStack()
    with stage_a:
        tp_ps = stage_a.enter_context(
            tc.tile_pool(name="tp_ps", bufs=3, space="PSUM"))
        pb_ps = stage_a.enter_context(
            tc.tile_pool(name="pb_ps", bufs=5, space="PSUM"))
        pb = stage_a.enter_context(tc.tile_pool(name="pb", bufs=3))

        def stats_tile(i):
            """sum(x^2) -> rstd_v = 1/(32*sqrt(mean+eps)) (ACT/DVE only)."""
            sq = pb.tile([P, D], F32, tag="sq")
            ssq = sstat.tile([P, 1], F32, tag="ssq")
            nc.scalar.activation(sq, xbf[:, i, :], AF.Square, accum_out=ssq)
            stdv = sstat.tile([P, 1], F32, tag="std")
            nc.scalar.activation(stdv, ssq, AF.Sqrt, bias=epsv_sb,
                                 scale=S_WQKV * S_WQKV / D)
            rstd_v = sstat.tile([P, 1], F32, tag="rstdv")
            nc.vector.reciprocal(rstd_v, stdv)
            return rstd_v

        def emit_k_tp(kt, khat):
            # eviction applies k_norm_w (per feature = per partition here)
            pt = tp_ps.tile([P, 2, P], BF16, tag="tp")
            for kf in range(2):
                nc.tensor.transpose(pt[:, kf, :],
                                    khat[:, kf * P:(kf + 1) * P], identity)
            nc.vector.tensor_scalar_mul(
                kT2[:, :, kt * P:(kt + 1) * P], pt, kw2_sb)

        def emit_q_tp(qt, qhats):
            for half in range(2):
                for j in range(0, 4, 2):
                    pt = tp_ps.tile([P, 2, P], BF16, tag="tp")
                    nc.tensor.transpose(
                        pt[:, 0, :], qhats[half][:, j * P:(j + 1) * P],
                        identity)
                    nc.tensor.transpose(
                        pt[:, 1, :], qhats[half][:, (j + 1) * P:(j + 2) * P],
                        identity)
                    nc.vector.tensor_scalar_mul(
                        qkT0[0:HD, half * 4 + j:half * 4 + j + 2,
                             qt * P:(qt + 1) * P], pt[0:HD], qw2_sb[0:HD])
                    nc.vector.tensor_scalar_mul(
                        qkT1[HD:P, half * 4 + j:half * 4 + j + 2,
                             qt * P:(qt + 1) * P], pt[HD:P], qw2_sb[HD:P])

        # kq-hat transposes run TWO tiles behind their matmuls so the
        # qk-norm ACT/DVE chains never stall the PE stream.
        rstds = {0: stats_tile(0), 1: stats_tile(1)}
        pend_k = {}
        pend_q = {}
        for i in range(NKT):
            rstd_v = rstds.pop(i)
            # K / V projection for ctx tile i (fp8 DoubleRow over kd pairs)
            ps = pb_ps.tile([P, 512], F32, tag="ps")
            for j in range(KD // 2):
                nc.tensor.matmul(
                    ps, lhsT=xT[:, 2 * j:2 * j + 2, i * P:(i + 1) * P],
                    rhs=wkv_sb[:, 2 * j:2 * j + 2, :],
                    start=(j == 0), stop=(j == KD // 2 - 1), perf_mode=DR)
            kv_ps = ps

            # Q projection for own tile qt = i - 4
            q_pss = None
            if i >= NQT:
                qt = i - NQT
                col = OWN + qt * P
                q_pss = []
                for half in range(2):
                    ps = pb_ps.tile([P, 512], F32, tag="ps")
                    q_pss.append(ps)
                    for j in range(KD // 2):
                        nc.tensor.matmul(
                            ps, lhsT=xT[:, 2 * j:2 * j + 2, col:col + P],
                            rhs=wq_sb[:, 2 * j:2 * j + 2,
                                      half * 512:(half + 1) * 512],
                            start=(j == 0), stop=(j == KD // 2 - 1),
                            perf_mode=DR)

            # deep-behind transposes keep the PE stream dense
            if i - 4 in pend_k:
                emit_k_tp(i - 4, pend_k.pop(i - 4))
            if i - 3 - NQT in pend_q:
                emit_q_tp(i - 3 - NQT, pend_q.pop(i - 3 - NQT))

            # k-chain + v eviction (ACT/DVE)
            ps = kv_ps
            sqk = pb.tile([P, KC], F32, tag="sqk")
            nc.scalar.activation(sqk, ps[:, 0:KC], AF.Square)
            msk = pb.tile([P, KVH], F32, tag="msk")
            nc.vector.reduce_sum(
                msk, sqk.rearrange("p (h e) -> p h e", e=HD),
                axis=mybir.AxisListType.X)
            sck = sstat.tile([P, KVH], F32, tag="sck")
            nc.scalar.activation(sck, msk, AF.Sqrt, bias=qkeps_sb, scale=1.0 / HD)
            rck = sstat.tile([P, KVH], F32, tag="rck")
            nc.vector.reciprocal(rck, sck)
            khat = pb.tile([P, KC], BF16, tag="khat", bufs=5)
            nc.vector.tensor_mul(
                khat.rearrange("p (h e) -> p h e", e=HD),
                ps[:, 0:KC].rearrange("p (h e) -> p h e", e=HD),
                rck[:, :, None].broadcast_to([P, KVH, HD]))
            pend_k[i] = khat
            nc.scalar.activation(
                v65[:, i, :, 0:HD],
                ps[:, KC:2 * KC].rearrange("p (h e) -> p h e", e=HD),
                AF.Copy, scale=rstd_v)

            # q-chain
            if q_pss is not None:
                qt = i - NQT
                msq = pb.tile([P, H], F32, tag="msq")
                for half in range(2):
                    sqq = pb.tile([P, 512], F32, tag="sqq")
                    nc.scalar.activation(sqq, q_pss[half], AF.Square)
                    nc.vector.reduce_sum(
                        msq[:, half * 8:(half + 1) * 8],
                        sqq.rearrange("p (h e) -> p h e", e=HD),
                        axis=mybir.AxisListType.X)
                sc = sstat.tile([P, H], F32, tag="sc")
                nc.scalar.activation(sc, msq, AF.Sqrt, bias=qkeps_sb,
                                     scale=1.0 / HD)
                rc = sstat.tile([P, H], F32, tag="rc")
                nc.vector.reciprocal(rc, sc)
                qhats = []
                for half in range(2):
                    ps = q_pss[half]
                    qhat = pb.tile([P, 512], BF16, tag="qhat", bufs=4)
                    nc.vector.tensor_mul(
                        qhat.rearrange("p (h e) -> p h e", e=HD),
                        ps.rearrange("p (h e) -> p h e", e=HD),
                        rc[:, half * 8:(half + 1) * 8, None]
                        .broadcast_to([P, 8, HD]))
                    qhats.append(qhat)
                pend_q[qt] = qhats

            if i + 2 < NKT:
                rstds[i + 2] = stats_tile(i + 2)

        def warm_mm():
            # full-width matmul into a scratch PSUM tile: keeps the PE's
            # HAM activity window busy through transpose-only stretches
            # (transpose-mode does not count as PE-busy for the HAM)
            psd = pb_ps.tile([P, 512], F32, tag="ps")
            nc.tensor.matmul(psd, lhsT=identity, rhs=xbf[:, 0, 0:512],
                             start=True, stop=True)

        warm_mm()
        for i in sorted(pend_k):
            emit_k_tp(i, pend_k.pop(i))
            warm_mm()
        for qt in sorted(pend_q):
            emit_q_tp(qt, pend_q.pop(qt))
            warm_mm()

    # ---- Stage C: attention. Head pairs run on PE row groups 0/64. ----
    stage_c = ExitStack()
    with stage_c:
        es_pool = stage_c.enter_context(tc.tile_pool(name="es_pool", bufs=2))
        psc = stage_c.enter_context(
            tc.tile_pool(name="psc", bufs=1, space="PSUM"))
        ps_o = stage_c.enter_context(
            tc.tile_pool(name="ps_o", bufs=2, space="PSUM"))

        def emit_pv(h, eS, eSd):
            kvh = h // 4
            tau = (h % 4) + 4 * (h // 8)
            pi = ((h // 4) % 2)
            slot = 2 * tau + pi
            for qt in range(NQT):
                po = ps_o.tile([P, HD + 1], F32, tag="po")
                for j in range(5):
                    if j == 0:
                        lhs = eSd[:, qt, :]
                    else:
                        c = _es_col(qt, qt + j)
                        lhs = eS[:, c:c + P]
                    nc.tensor.matmul(
                        po, lhsT=lhs,
                        rhs=v65[:, qt + j, kvh, :],
                        start=(j == 0), stop=(j == 4))
                rec = sstat.tile([P, 1], F32, tag="rec")
                nc.vector.reciprocal(rec, po[:, HD:HD + 1])
                nc.vector.tensor_scalar_mul(
                    attn_sb[:, qt, slot * HD:(slot + 1) * HD], po[:, 0:HD],
                    rec)

        PAIRS = [(0, 4), (1, 5), (2, 6), (3, 7),
                 (8, 12), (9, 13), (10, 14), (11, 15)]
        pending = []
        for hA, hB in PAIRS:
            g = hA // 8
            tau = (hA % 4) + 4 * (hA // 8)
            eS_A = es_pool.tile([P, 2 * 1280], BF16, tag="esA")
            eS_B = es_pool.tile([P, 2 * 1280], BF16, tag="esB")
            eSd_A = es_pool.tile([P, NQT, P], BF16, tag="esdA")
            eSd_B = es_pool.tile([P, NQT, P], BF16, tag="esdB")
            for half in range(2):
                psA = psc.tile([P, 1280], F32, tag="psA")
                psB = psc.tile([P, 1280], F32, tag="psB")
                for ki in range(half * 4, half * 4 + 4):
                    qlo, qhi = _qclip(ki)
                    w = qhi - qlo
                    o = OFF[ki] - half * 1280
                    nc.tensor.matmul(
                        psA[:, o:o + w],
                        lhsT=kT2[:, g, ki * P:(ki + 1) * P],
                        rhs=qkT0[:, tau, qlo:qhi],
                        start=True, stop=True)
                    nc.tensor.matmul(
                        psB[:, o:o + w],
                        lhsT=kT2[:, g, ki * P:(ki + 1) * P],
                        rhs=qkT1[:, tau, qlo:qhi],
                        start=True, stop=True)
                for eS, psX in ((eS_A, psA), (eS_B, psB)):
                    nc.scalar.activation(
                        eS[:, half * 1280:(half + 1) * 1280], psX, AF.Exp,
                        scale=SM_SCALE)
                # diagonal-block masks: d0 via gpsimd into eSd (PV reads
                # eSd), d4 zeroed inside eS via a predicated write (DVE)
                for qt in range(NQT):
                    if half == 0:
                        c = _es_col(qt, qt)
                        for eS, eSd in ((eS_A, eSd_A), (eS_B, eSd_B)):
                            nc.gpsimd.tensor_mul(
                                eSd[:, qt, :], eS[:, c:c + P], tri_g)
                    else:
                        c = _es_col(qt, qt + 4)
                        for eS in (eS_A, eS_B):
                            nc.vector.copy_predicated(
                                eS[:, c:c + P], tri_p, zeros_sb)
                # previous pair's PV fills the PE while exp drains psA/psB
                if pending:
                    emit_pv(pending.pop(0), pending.pop(0), pending.pop(0))
            pending = [hA, eS_A, eSd_A, hB, eS_B, eSd_B]
        emit_pv(pending.pop(0), pending.pop(0), pending.pop(0))
        emit_pv(pending.pop(0), pending.pop(0), pending.pop(0))

    # ---- Stages D+E: attnT transpose + wo (fp8 DR) + residual + ffn norm ----
    attnT_stack = ExitStack()
    attnT_pool = attnT_stack.enter_context(tc.tile_pool(name="attnT_pool",
                                                        bufs=1))
    attnT = attnT_pool.tile([P, KD, OWN], FP8)

    stage_de = ExitStack()
    with stage_de:
        wo_pool = stage_de.enter_context(tc.tile_pool(name="wo_pool", bufs=1))
        wo_sb = wo_pool.tile([P, KD, D], FP8)
        for c in range(4):
            nc.sync.dma_start(
                wo_sb[:, 2 * c:2 * c + 2, :],
                io["woT"][2 * c * P:2 * (c + 1) * P, :]
                .rearrange("(kd p) n -> p kd n", p=P))
        ps_r = stage_de.enter_context(
            tc.tile_pool(name="ps_r", bufs=3, space="PSUM"))
        tp_d = stage_de.enter_context(
            tc.tile_pool(name="tp_d", bufs=3, space="PSUM"))
        tp_ps3 = stage_de.enter_context(
            tc.tile_pool(name="tp_ps3", bufs=2, space="PSUM"))
        pe = stage_de.enter_context(tc.tile_pool(name="pe", bufs=2))

        def emit_attnT(qt):
            for kd in range(0, KD, 2):
                pt = tp_d.tile([P, 2, P], BF16, tag="tpd")
                nc.tensor.transpose(pt[:, 0, :],
                                    attn_sb[:, qt, kd * P:(kd + 1) * P],
                                    identity)
                nc.tensor.transpose(pt[:, 1, :],
                                    attn_sb[:, qt, (kd + 1) * P:(kd + 2) * P],
                                    identity)
                nc.vector.tensor_copy(
                    attnT[:, kd:kd + 2, qt * P:(qt + 1) * P], pt)

        emit_attnT(0)
        emit_attnT(1)
        pend_hn = None
        for qt in range(NQT):
            xr = xown[:, qt, :]
            for half in range(2):
                ps = ps_r.tile([P, 512], F32, tag="psr")
                for j in range(KD // 2):
                    nc.tensor.matmul(
                        ps, lhsT=attnT[:, 2 * j:2 * j + 2, qt * P:(qt + 1) * P],
                        rhs=wo_sb[:, 2 * j:2 * j + 2,
                                  half * 512:(half + 1) * 512],
                        start=(j == 0), stop=(j == KD // 2 - 1), perf_mode=DR)
                nc.vector.scalar_tensor_tensor(
                    h_sb[:, qt, half * 512:(half + 1) * 512], ps, io["c_wo"],
                    xr[:, half * 512:(half + 1) * 512],
                    op0=mybir.AluOpType.mult, op1=mybir.AluOpType.add)
            psd = ps_r.tile([P, 512], F32, tag="psr")
            nc.tensor.matmul(psd, lhsT=identity, rhs=attn_sb[:, 0, 0:512],
                             start=True, stop=True)
            if qt + 2 < NQT:
                emit_attnT(qt + 2)
            if pend_hn is not None:
                pqt, phn = pend_hn
                for kd in range(0, KD, 2):
                    pt = tp_ps3.tile([P, 2, P], BF16, tag="tp3")
                    nc.tensor.transpose(pt[:, 0, :],
                                        phn[:, kd * P:(kd + 1) * P], identity)
                    nc.tensor.transpose(pt[:, 1, :],
                                        phn[:, (kd + 1) * P:(kd + 2) * P],
                                        identity)
                    nc.vector.tensor_copy(
                        hnT[:, kd:kd + 2, pqt * P:(pqt + 1) * P], pt)
            # ffn rmsnorm; hn is scaled by 1/8 to undo the w1/w3 fp8 scale
            sqh = pe.tile([P, D], F32, tag="sqh")
            ssqh = sstat.tile([P, 1], F32, tag="ssq")
            nc.scalar.activation(sqh, h_sb[:, qt, :], AF.Square, accum_out=ssqh)
            stdh = sstat.tile([P, 1], F32, tag="std")
            nc.scalar.activation(stdh, ssqh, AF.Sqrt, bias=epsh_sb,
                                 scale=S_W13 * S_W13 / D)
            rstdh = sstat.tile([P, 1], F32, tag="rstd")
            nc.vector.reciprocal(rstdh, stdh)
            hn = pe.tile([P, D], BF16, tag="hn")
            nc.vector.tensor_scalar_mul(hn, h_sb[:, qt, :], rstdh)
            pend_hn = (qt, hn)
        pqt, phn = pend_hn
        for kd in range(0, KD, 2):
            pt = tp_ps3.tile([P, 2, P], BF16, tag="tp3")
            nc.tensor.transpose(pt[:, 0, :], phn[:, kd * P:(kd + 1) * P],
                                identity)
            nc.tensor.transpose(pt[:, 1, :], phn[:, (kd + 1) * P:(kd + 2) * P],
                                identity)
            nc.vector.tensor_copy(hnT[:, kd:kd + 2, pqt * P:(pqt + 1) * P],
                                  pt)

    attnT_stack.close()
    xall_stack.close()
    ap_stack.close()

    # ---- Stage F: SwiGLU FFN (fp8 DoubleRow) ----
    stage_f = ExitStack()
    with stage_f:
        fT_pool = stage_f.enter_context(tc.tile_pool(name="fT_pool", bufs=1))
        fT = fT_pool.tile([P, KH, OWN], FP8)   # silu(g) * u, feature-major
        w13 = stage_f.enter_context(tc.tile_pool(name="w13", bufs=4))
        ps_f = stage_f.enter_context(
            tc.tile_pool(name="ps_f", bufs=2, space="PSUM"))
        pf = stage_f.enter_context(tc.tile_pool(name="pf", bufs=2))

        for mi in range(KH):
            w1t = w13.tile([P, KD, P], FP8, tag="w1t")
            nc.sync.dma_start(
                w1t, io["w1T"][:, mi * P:(mi + 1) * P]
                .rearrange("(kd p) m -> p kd m", p=P))
            w3t = w13.tile([P, KD, P], FP8, tag="w3t")
            nc.sync.dma_start(
                w3t, io["w3T"][:, mi * P:(mi + 1) * P]
                .rearrange("(kd p) m -> p kd m", p=P))
            psg = ps_f.tile([P, 512], F32, tag="pg")
            for j in range(KD // 2):
                nc.tensor.matmul(psg, lhsT=w1t[:, 2 * j:2 * j + 2, :],
                                 rhs=hnT[:, 2 * j:2 * j + 2, :],
                                 start=(j == 0), stop=(j == KD // 2 - 1),
                                 perf_mode=DR)
            psu = ps_f.tile([P, 512], F32, tag="pu")
            for j in range(KD // 2):
                nc.tensor.matmul(psu, lhsT=w3t[:, 2 * j:2 * j + 2, :],
                                 rhs=hnT[:, 2 * j:2 * j + 2, :],
                                 start=(j == 0), stop=(j == KD // 2 - 1),
                                 perf_mode=DR)
            # psg/psu are exact g/u (scales cancelled): silu via sigmoid
            sg = pf.tile([P, 512], F32, tag="sg")
            nc.scalar.activation(sg, psg, AF.Sigmoid)
            gm = pf.tile([P, 512], F32, tag="gm")
            nc.vector.tensor_mul(gm, sg, psg)
            nc.vector.tensor_mul(fT[:, mi, :], gm, psu)

        w2_pool = stage_f.enter_context(tc.tile_pool(name="w2_pool", bufs=1))
        w2_sb = w2_pool.tile([P, KH, D], FP8)
        for c in range(8):
            nc.sync.dma_start(
                w2_sb[:, c * 4:(c + 1) * 4, :],
                io["w2T"][c * 4 * P:(c + 1) * 4 * P, :]
                .rearrange("(kh p) n -> p kh n", p=P))
        ps_y = stage_f.enter_context(
            tc.tile_pool(name="ps_y", bufs=2, space="PSUM"))
        py = stage_f.enter_context(tc.tile_pool(name="py", bufs=2))

        for qt in range(NQT):
            yt = py.tile([P, D], F32, tag="yt")
            for half in range(2):
                ps = ps_y.tile([P, 512], F32, tag="psy")
                for j in range(KH // 2):
                    nc.tensor.matmul(
                        ps, lhsT=fT[:, 2 * j:2 * j + 2, qt * P:(qt + 1) * P],
                        rhs=w2_sb[:, 2 * j:2 * j + 2,
                                  half * 512:(half + 1) * 512],
                        start=(j == 0), stop=(j == KH // 2 - 1), perf_mode=DR)
                # undo the w2 fp8 range scale and apply ffn_scale's scalar
                nc.vector.scalar_tensor_tensor(
                    yt[:, half * 512:(half + 1) * 512], ps, io["c_y"],
                    h_sb[:, qt, half * 512:(half + 1) * 512],
                    op0=mybir.AluOpType.mult, op1=mybir.AluOpType.add)
            nc.sync.dma_start(y[qt * P:(qt + 1) * P, :], yt)


def build_nc(c_y: float, c_wo: float):
    nc = bacc.Bacc("TRN2", target_bir_lowering=False, debug=False,
                   num_devices=NCORES)
    io = {
        "xbf": nc.dram_tensor("xbf", [CTX, D], BF16, kind="ExternalInput").ap(),
        "xTf8": nc.dram_tensor("xTf8", [D, CTX], FP8,
                               kind="ExternalInput").ap(),
        "xown": nc.dram_tensor("xown", [OWN, D], F32,
                               kind="ExternalInput").ap(),
        "wqT": nc.dram_tensor("wqT", [D, D], FP8, kind="ExternalInput").ap(),
        "wkvT": nc.dram_tensor("wkvT", [D, 2 * KVH * HD], FP8,
                               kind="ExternalInput").ap(),
        "woT": nc.dram_tensor("woT", [D, D], FP8, kind="ExternalInput").ap(),
        "w1T": nc.dram_tensor("w1T", [D, HID], FP8, kind="ExternalInput").ap(),
        "w3T": nc.dram_tensor("w3T", [D, HID], FP8, kind="ExternalInput").ap(),
        "w2T": nc.dram_tensor("w2T", [HID, D], FP8, kind="ExternalInput").ap(),
        "qw2": nc.dram_tensor("qw2", [P, 1], F32, kind="ExternalInput").ap(),
        "kw2": nc.dram_tensor("kw2", [P, 1], F32, kind="ExternalInput").ap(),
        "tri_g": nc.dram_tensor("tri_g", [P, P], BF16,
                                kind="ExternalInput").ap(),
        "tri_p": nc.dram_tensor("tri_p", [P, P], mybir.dt.uint8,
                                kind="ExternalInput").ap(),
        "vones": nc.dram_tensor("vones", [P, NKT], BF16,
                                kind="ExternalInput").ap(),
        "y": nc.dram_tensor("y", [OWN, D], F32, kind="ExternalOutput").ap(),
        "c_y": c_y,
        "c_wo": c_wo,
    }
    with tile.TileContext(nc) as tc:
        with ExitStack() as ctx:
            _build_tile_kernel(ctx, tc, io)
    nc.compile()
    return nc


_CACHE = {}


def get_nc(c_y: float, c_wo: float):
    if "nc" not in _CACHE:
        _CACHE["nc"] = build_nc(c_y, c_wo)
    return _CACHE["nc"]


def _fp8(a):
    return np.ascontiguousarray(
        np.clip(a, -240.0, 240.0)).astype(NPFP8)


def prep_in_maps(inputs):
    """Fold scales into weights, transpose/cast, and slice per-core inputs."""
    f32 = np.float32
    x = np.asarray(inputs["x"], f32)
    wq = np.asarray(inputs["wq"], f32)
    wk = np.asarray(inputs["wk"], f32)
    wv = np.asarray(inputs["wv"], f32)
    wo = np.asarray(inputs["wo"], f32)
    w1 = np.asarray(inputs["w1"], f32)
    w2 = np.asarray(inputs["w2"], f32)
    w3 = np.asarray(inputs["w3"], f32)
    qw = np.asarray(inputs["q_norm_w"], f32)
    kw = np.asarray(inputs["k_norm_w"], f32)
    anw = np.asarray(inputs["attn_norm_w"], f32)
    fnw = np.asarray(inputs["ffn_norm_w"], f32)
    asc = np.asarray(inputs["attn_scale"], f32)
    fsc = np.asarray(inputs["ffn_scale"], f32)

    HEAD_PERM = [0, 4, 1, 5, 2, 6, 3, 7, 8, 12, 9, 13, 10, 14, 11, 15]
    wq_p = (wq * anw[None, :]).reshape(H, HD, D)[HEAD_PERM].reshape(H * HD, D)
    wqT = _fp8(wq_p.T * S_WQKV)
    wkvT = _fp8(
        np.concatenate([wk * anw[None, :], wv * anw[None, :]], axis=0).T
        * S_WQKV)
    asc_s = float(np.mean(asc))
    c_wo = asc_s / S_WO
    wo_p = ((wo * (asc / np.float32(asc_s))[:, None])
            .T.reshape(H, HD, D)[HEAD_PERM].reshape(H * HD, D))
    woT = _fp8(wo_p * S_WO)
    w1T = _fp8((w1 * fnw[None, :]).T * S_W13)
    w3T = _fp8((w3 * fnw[None, :]).T * S_W13)
    fsc_s = float(np.mean(fsc))
    c_y = fsc_s / S_W2
    w2T = _fp8((w2 * (fsc / np.float32(fsc_s))[:, None]).T * S_W2)
    qwb = np.ascontiguousarray(np.tile(qw, 2)[:, None]).astype(f32)
    kwb = np.ascontiguousarray(np.tile(kw, 2)[:, None]).astype(f32)

    # diagonal-block triangle masks:
    # d0 block (ki==qt): valid iff k > qq (bf16 VALID mask, gpsimd multiply)
    # d4 block (ki==qt+4): valid iff k <= qq (uint8 INVALID mask, DVE zeroing)
    k_i = np.arange(P)[:, None]
    q_i = np.arange(P)[None, :]
    tri_g = np.ascontiguousarray((k_i > q_i).astype(NPBF16))
    tri_p = np.ascontiguousarray((k_i > q_i).astype(np.uint8))

    # per-token validity for V's appended column (0 for halo padding)
    v_int = np.ones((P, NKT), NPBF16)
    v_first = np.zeros((P, NKT), NPBF16)
    v_first[:, NQT:] = 1.0

    shared = dict(wqT=wqT, wkvT=wkvT, woT=woT, w1T=w1T, w3T=w3T, w2T=w2T,
                  qw2=qwb, kw2=kwb, tri_g=tri_g, tri_p=tri_p)
    in_maps = []
    for b in range(B):
        for j in range(T // OWN):
            xc = np.zeros((CTX, D), f32)
            if j == 0:
                xc[OWN:] = x[b, 0:OWN]
                vm = v_first
            else:
                xc[:] = x[b, (j - 1) * OWN:(j + 1) * OWN]
                vm = v_int
            in_maps.append(dict(xbf=xc.astype(NPBF16),
                                xTf8=_fp8(np.ascontiguousarray(xc.T)),
                                xown=np.ascontiguousarray(xc[OWN:]),
                                vones=vm, **shared))
    return in_maps, c_y, c_wo


LAST_RESULTS = None


def _ensure_ntff_hook():
    """Install the axon NTFF profile hook if the image's antenv lacks it."""
    import types
    try:
        from antenv.axon_hooks import get_axon_ntff_profile_hook  # noqa: F401
        return  # real module present
    except ImportError:
        pass
    try:
        import antenv
        boot_dir = "/root/.axon_site/trn_agent_boot"
        if boot_dir not in sys.path:
            sys.path.insert(0, boot_dir)
        import trn_boot
        hook = trn_boot._ntff_profile_via_ctypes("/opt/axon/libaxon_pjrt.so")
        mod = types.ModuleType("antenv.axon_hooks")
        mod._hook = hook
        mod.get_axon_ntff_profile_hook = lambda: mod._hook
        mod.set_axon_ntff_profile_hook = lambda h: setattr(mod, "_hook", h)
        sys.modules["antenv.axon_hooks"] = mod
        antenv.axon_hooks = mod
        import concourse.bass_utils as _bu
        _bu.upload_artifacts = lambda tmpdir: tmpdir
    except Exception as e:  # pragma: no cover
        print(f"ntff hook unavailable ({e}); running without trace")


def kernel(**inputs):
    global LAST_RESULTS
    if os.environ.get("BASS_TRACE"):
        _ensure_ntff_hook()
    in_maps, c_y, c_wo = prep_in_maps(inputs)
    nc = get_nc(c_y, c_wo)
    res = run_bass_kernel_spmd(nc, in_maps, core_ids=list(range(NCORES)))
    LAST_RESULTS = res
    y = np.empty((B, T, D), np.float32)
    for c in range(NCORES):
        b, j = divmod(c, T // OWN)
        y[b, j * OWN:(j + 1) * OWN] = res.results[c]["y"]
    return y
